# revision 19
# baseline (speedup 1.0000x reference)
"""Multi-head attention (RoPE, causal) Trainium2 Bass kernel.

Sharding (8 cores): data-parallel over batch (4) x tensor-parallel over
heads (16 -> 2 groups of 8).  Core c handles batch c//2 and head group
c%2.  Attention is fully head-local; the out-projection partial sums of
the two head groups of each batch are added on the host.

Per-core device kernel (sizes hardcoded for b=4, n=2048, hidden=1024,
h=16, d=64).  All matmuls in bf16 with fp32 PSUM accumulation (an fp8
DoubleRow variant was 2x faster on the PE but exceeded the 2e-2 error
budget; any single fp8 stage already costs >=4e-2 max-rel error):
  - QKV projections as bf16 matmuls; x and W arrive host-pre-transposed
    (c-major) and pre-cast, so no on-device transposes.
  - RoPE in head-transposed layout: u = q*sinP on DVE (sinP pair-swapped
    on the host so the +-1 pair-permutation matmul PM @ u on the PE
    directly yields rot(q)*sin), then q*cos + that on DVE; all DVE
    operands are bf16 SBUF so the 2x mode applies.
  - Scores computed transposed, s_T[k, q], two heads of a pair row-tiled
    onto the PE at tile_position (0,0)/(64,0), into one [128, 2, 512]
    PSUM tile so a single Exp activation (scale=1/8 folded in) covers
    both heads.  Diagonal blocks compute only the valid q column range
    [vs:512); the <=128-wide partial triangle is zeroed by a bf16 mask
    multiply on the otherwise-idle GpSimd engine.
  - AV: v augmented with 64 pad-ones columns (M=128) so PSUM rows
    64:128 hold the softmax denominator already broadcast across 64
    partitions; normalization is reciprocal+mul on DVE.
  - Software-pipelined strips: the projections of strip ic+1 are
    interleaved between the four attention pairs of strip ic so the Act
    engine (exp) never drains; the out-projection is deferred one strip
    so DVE normalization never stalls the PE.
"""

from itertools import zip_longest

import numpy as np
import ml_dtypes

import concourse.bass as bass
import concourse.mybir as mybir
from concourse import bacc
from concourse import hw_specs as _hw_specs
from concourse.tile import TileContext
from concourse.bass_utils import run_bass_kernel_spmd

# Calibrate the Tile scheduler's cost model to rates measured on this HW
# (microbenchmarks, steady-state chains): bf16 matmuls stream ~2 columns
# per cycle (F=512 chain ~150 ns vs the 213 ns 1-col/cycle default), the
# Act exp and DVE ops run ~0.7-0.8x the default cost, and GpSimd 2-input
# ops are ~2x SLOWER than the default.  A closer model gives the static
# per-engine schedule fewer runtime stalls.  Must run before the first
# compile in the process: the Rust cost model snapshots these class attrs
# into a process-global OnceCell on first use.
_hw_specs.TRN2Spec.PE_CYCLE = 1e9 / 3.2e9
_hw_specs.TRN2Spec.PE_CYCLE_PSTATE_MID = 1e9 / 1.6e9
_hw_specs.TRN2Spec.CYCLE_T = {
    **_hw_specs.TRN2Spec.CYCLE_T,
    mybir.EngineType.DVE: 0.60,
    mybir.EngineType.Activation: 0.625,
    mybir.EngineType.Pool: 1.67,
}

# ---------------------------------------------------------------- constants
B, N, HID = 4, 2048, 1024
H = 16
D = HID // H                     # 64
NCORES = 8
GROUPS = NCORES // B             # 2 head groups
HPG = H // GROUPS                # 8 heads per core
HD = HPG * D                     # 512 local head dims
PAIRS = HPG // 2                 # 4 head pairs per core
ROPE_THETA = 10000.0
SCALE = 0.125                    # 1/sqrt(d)
WSCALE = 1.0

P = 128
CC = HID // P                    # 8 contraction chunks for projections
ICH = 512                        # projection i-chunk (moving free dim)
QCH = 512                        # attention q-chunk
KCH = 128                        # attention k-chunk
NQC = N // QCH                   # 4
NKC = N // KCH                   # 16

F32 = mybir.dt.float32
BF16 = mybir.dt.bfloat16
FP8 = mybir.dt.float8e4
BF16NP = ml_dtypes.bfloat16
FP8NP = ml_dtypes.float8_e4m3
DR = mybir.MatmulPerfMode.DoubleRow

_NC_CACHE = {}
MAX_STRIPS = None
REPEAT = 1


# ---------------------------------------------------------------- host prep
def _allow_matrix(is_causal, start_pos):
    i = np.arange(N)[:, None]    # query index
    j = np.arange(N)[None, :]    # key index
    if is_causal:
        return (j < start_pos) | ((i >= start_pos) & (i >= j))
    return np.ones((N, N), dtype=bool)


def _block_plan(is_causal, start_pos):
    """Classify each (qc, kc) score block.

    plan[(qc, kc)] is (kind, var, vs, ms, me):
      kind: 'skip' | 'full' | 'partial'
      var:  mask variant index (partial only)
      vs:   first valid q column in the 512-wide chunk (block computed
            over [vs:512) only)
      [ms, me): q column range where the partial triangle needs masking
    """
    allow = _allow_matrix(is_causal, start_pos)
    plan = {}
    variants = []
    vkeys = {}
    for qc in range(NQC):
        for kc in range(NKC):
            blk = allow[qc * QCH:(qc + 1) * QCH, kc * KCH:(kc + 1) * KCH]
            if not blk.any():
                plan[(qc, kc)] = ("skip", None, 0, 0, 0)
                continue
            if blk.all():
                plan[(qc, kc)] = ("full", None, 0, 0, 0)
                continue
            bT = blk.T               # [128 k, 512 q]
            start = np.argmax(bT, axis=1)
            for r in range(KCH):
                if not bT[r].any():
                    raise NotImplementedError("empty k-row in partial block")
                s = start[r]
                if not bT[r, s:].all() or bT[r, :s].any():
                    raise NotImplementedError("non-suffix mask row")
            key = start.tobytes()
            if key not in vkeys:
                vkeys[key] = len(variants)
                variants.append(start.astype(np.float32))
            vs = int(start.min())
            me = int(start.max())
            plan[(qc, kc)] = ("partial", vkeys[key], vs, vs, me)
    if not variants:
        variants.append(np.zeros(KCH, dtype=np.float32))
    # sanity: first non-skip block per qc row must cover the full q range
    for qc in range(NQC):
        for kc in range(NKC):
            kind, _, vs, _, _ = plan[(qc, kc)]
            if kind != "skip":
                assert vs == 0, f"first block of row {qc} has vs={vs}"
                break
    # If every variant is a slope-1 diagonal start[k] = k + o (the causal
    # case), all masks are column slices of ONE [128, QCH + o_max] ramp
    # R[k, t] = (t >= k + o_max): variant o lives at column offset
    # o_max - o.  That shrinks the SBUF table ~2.3x vs concatenation.
    offs = []
    for v in variants:
        o = int(v[0])
        if np.array_equal(v, np.arange(KCH) + o):
            offs.append(o)
        else:
            offs.append(None)
    if all(o is not None for o in offs) and variants:
        o_max = max(offs)
        k = np.arange(KCH)[:, None]
        t = np.arange(QCH + o_max)[None, :]
        masks = (t >= k + o_max).astype(np.float32)   # [128, QCH + o_max]
        for key in plan:
            kind, var, vs, ms, me = plan[key]
            if kind == "partial":
                plan[key] = (kind, o_max - offs[var], vs, ms, me)
    else:
        # fallback: concatenated per-variant blocks at offsets var*QCH
        q = np.arange(QCH)[None, :]
        blocks = [(q >= v[:, None]).astype(np.float32) for v in variants]
        masks = np.concatenate(blocks, axis=1)        # [128, V*QCH]
        for key in plan:
            kind, var, vs, ms, me = plan[key]
            if kind == "partial":
                plan[key] = (kind, var * QCH, vs, ms, me)
    return plan, masks


def _rope_tables():
    inv_freq = 1.0 / (ROPE_THETA ** (np.arange(0, D, 2, dtype=np.float64) / D))
    t = np.arange(N, dtype=np.float64)
    freqs = t[:, None] * inv_freq[None, :]        # [N, 32]
    freqs = np.repeat(freqs, 2, axis=1)           # [N, 64]
    cos = np.cos(freqs).T.astype(np.float32)      # [64, N]
    sin = np.sin(freqs).T.astype(np.float32)
    # sinN folds the rotate-half signs: dst = raw*cos + pairswap(raw)*sinN
    # with sinN[2r] = -sin[2r], sinN[2r+1] = +sin[2r+1]; the pair swap is a
    # partition-strided SBUF->SBUF DMA so no PE/PSUM is involved.
    sinN = sin.copy()
    sinN[0::2] = -sin[0::2]
    # duplicate rows so both heads of a pair (partitions 0:64 / 64:128)
    # see the same table at matching partition base
    cos2 = np.concatenate([cos, cos], axis=0)     # [128, N]
    sin2 = np.concatenate([sinN, sinN], axis=0)
    return np.ascontiguousarray(cos2), np.ascontiguousarray(sin2)


# ---------------------------------------------------------------- device IR
def _build_nc(is_causal, start_pos):
    plan, masks_np = _block_plan(is_causal, start_pos)
    mwid = masks_np.shape[1]
    streaming = bool(is_causal)
    exp_scale = SCALE

    nc = bacc.Bacc("TRN2", target_bir_lowering=False, debug=False)

    xqT = nc.declare_dram_parameter("xqT", [HID, N], BF16, isOutput=False).ap()
    xkT = nc.declare_dram_parameter("xkT", [HID, N], BF16, isOutput=False).ap()
    xvT = nc.declare_dram_parameter("xvT", [HID, N], BF16, isOutput=False).ap()
    wqT = nc.declare_dram_parameter("wqT", [HID, HD], BF16, isOutput=False).ap()
    wkT = nc.declare_dram_parameter("wkT", [HID, HD], BF16, isOutput=False).ap()
    wvT = nc.declare_dram_parameter("wvT", [HID, HD], BF16, isOutput=False).ap()
    woT = nc.declare_dram_parameter("woT", [HD, HID], BF16, isOutput=False).ap()
    cos_d = nc.declare_dram_parameter("cos", [P, N], BF16, isOutput=False).ap()
    sin_d = nc.declare_dram_parameter("sin", [P, N], BF16, isOutput=False).ap()
    msk_d = nc.declare_dram_parameter("masks", [P, mwid], BF16,
                                      isOutput=False).ap()
    kpad_d = nc.declare_dram_parameter("kpad", [P, NKC], BF16, isOutput=False).ap()
    y = nc.declare_dram_parameter("y", [N, HID], F32, isOutput=True).ap()

    with TileContext(nc) as tc:
        with (
            tc.tile_pool(name="const", bufs=1) as const,
            tc.tile_pool(name="persist", bufs=1) as persist,
            tc.tile_pool(name="xstrip", bufs=3) as xpool,
            tc.tile_pool(name="qpool", bufs=2) as qpool,
            tc.tile_pool(name="aopool", bufs=2) as aopool,
            tc.tile_pool(name="work", bufs=2) as work,
            tc.tile_pool(name="ppool", bufs=6) as ppool,
            tc.tile_pool(name="psmm", bufs=2, space="PSUM") as psmm,
            tc.tile_pool(name="pssc", bufs=2, space="PSUM") as pssc,
            tc.tile_pool(name="psav", bufs=2, space="PSUM") as psav,
        ):
            # ---------------- weights first: the first projections need them
            wv = const.tile([P, CC, HD], BF16, tag="wv", name="wv")
            nc.sync.dma_start(out=wv, in_=wvT.rearrange("(cc p) m -> p cc m", p=P))
            wq = const.tile([P, CC, HD], BF16, tag="wq", name="wq")
            nc.sync.dma_start(out=wq, in_=wqT.rearrange("(cc p) m -> p cc m", p=P))

            # strip-0 x loads, issued before the rest of the tables
            def load_strip(x_dram, ic, nm, split=1):
                strip = xpool.tile([P, CC, ICH], BF16, tag="xstrip", name=nm)
                src = x_dram.rearrange("(cc p) n -> p cc n", p=P)[
                    :, :, ic * ICH:(ic + 1) * ICH]
                step = CC // split
                for s in range(split):
                    nc.sync.dma_start(out=strip[:, s * step:(s + 1) * step],
                                      in_=src[:, s * step:(s + 1) * step])
                return strip

            strip0_v = load_strip(xvT, 0, "strip0v")
            strip0_q = load_strip(xqT, 0, "strip0q")
            wk = const.tile([P, CC, HD], BF16, tag="wk", name="wk")
            nc.sync.dma_start(out=wk, in_=wkT.rearrange("(cc p) m -> p cc m", p=P))
            strip0_k = load_strip(xkT, 0, "strip0k")

            # ---------------- remaining constants / tables
            cost = const.tile([P, N], BF16, tag="cos", name="cos")
            sint = const.tile([P, N], BF16, tag="sin", name="sin")
            nc.sync.dma_start(out=cost, in_=cos_d)
            nc.sync.dma_start(out=sint, in_=sin_d)
            mskt = const.tile([P, mwid], BF16, tag="masks", name="mskt")
            nc.sync.dma_start(out=mskt, in_=msk_d)
            kpad = const.tile([P, NKC], BF16, tag="kpad", name="kpad")
            nc.sync.dma_start(out=kpad, in_=kpad_d)
            wo = const.tile([P, PAIRS, HID], BF16, tag="wo", name="wo")
            nc.sync.dma_start(out=wo, in_=woT.rearrange("(jc p) o -> p jc o", p=P))

            # ---------------- persistent activations
            # roped k per pair, double-buffered across repeats so the
            # next repeat's projections never wait on this one's attention
            kTs = [[persist.tile([P, N], BF16, tag=f"kT{s}{p}",
                                 name=f"kT{s}{p}")
                    for p in range(PAIRS)] for s in range(2)]
            kT = kTs[0]
            # v: per (head, k-chunk) blocks of [v(64) | pad-ones(64)]
            vtalls = [persist.tile([P, HPG, NKC, P], BF16, tag=f"vt{s}",
                                   name=f"vt{s}") for s in range(2)]
            vts = [[va[:, h] for h in range(HPG)] for va in vtalls]
            vtall, vt = vtalls[0], vts[0]
            if streaming:
                qT = None
            else:
                qT = [persist.tile([P, N], BF16, tag=f"qT{p}", name=f"qT{p}")
                      for p in range(PAIRS)]

            # pad-ones halves of vt, written once on the (idle) gpsimd engine
            for va in vtalls:
                for kc in range(NKC):
                    nc.gpsimd.tensor_copy(
                        out=va[:, :, kc, D:P],
                        in_=kpad[:, kc:kc + 1].to_broadcast([P, HPG, D]),
                    )

            def proj_qk_mc(w_sb, ic, mc, dst_of_mc, strip):
                """One q/k projection chain (pair mc of a 512-wide strip)."""
                isl = slice(ic * ICH, (ic + 1) * ICH)
                ps = psmm.tile([P, ICH], F32, tag="mm", name="pjmm")
                for cc in range(CC):
                    nc.tensor.matmul(
                        ps,
                        lhsT=w_sb[:, cc, mc * P:(mc + 1) * P],
                        rhs=strip[:, cc, :],
                        start=(cc == 0),
                        stop=(cc == CC - 1),
                    )
                raw = work.tile([P, ICH], BF16, tag="raw", name="raw")
                nc.vector.tensor_copy(out=raw, in_=ps)
                # rotate-half as a partition pair swap on the (idle) DMA
                # engines; the signs live in the sinN table
                rsw = work.tile([P, ICH], BF16, tag="rsw", name="rsw")
                nc.sync.dma_start(out=rsw[0:P - 1:2], in_=raw[1:P:2])
                nc.sync.dma_start(out=rsw[1:P:2], in_=raw[0:P - 1:2])
                u = work.tile([P, ICH], BF16, tag="ropeu", name="u")
                nc.vector.tensor_mul(u, rsw, sint[:, isl])
                cw = work.tile([P, ICH], BF16, tag="ropecw", name="cw")
                nc.vector.tensor_mul(cw, raw, cost[:, isl])
                nc.vector.tensor_add(dst_of_mc(mc), cw, u)

            def proj_qk_strip(x_dram, w_sb, ic, dst_of_mc, strip=None):
                """Project one 512-wide strip of q or k (all pairs) + RoPE."""
                if strip is None:
                    strip = load_strip(x_dram, ic, "strip")
                for mc in range(PAIRS):
                    proj_qk_mc(w_sb, ic, mc, dst_of_mc, strip)

            def proj_v_sub(ic4, sub, strip, vset):
                """Project one 128-wide sub-chunk of a v strip into vt."""
                vta = vtalls[vset]
                ic = ic4 * (ICH // P) + sub
                ps = psmm.tile([P, HD], F32, tag="mm", name="pvmm")
                for cc in range(CC):
                    nc.tensor.matmul(
                        ps,
                        lhsT=strip[:, cc, sub * P:(sub + 1) * P],
                        rhs=wv[:, cc, :],
                        start=(cc == 0),
                        stop=(cc == CC - 1),
                    )
                padb = kpad[:, ic:ic + 1]
                nc.vector.tensor_mul(
                    vta[:, :, ic, 0:D],
                    ps.rearrange("p (h d) -> p h d", h=HPG),
                    padb.to_broadcast([P, HPG, D]),
                )

            def proj_v_strip(ic4, strip=None, vset=0):
                """Project one 512-wide strip of v into vt (pad-scaled)."""
                if strip is None:
                    strip = load_strip(xvT, ic4, "vstrip")
                for sub in range(ICH // P):
                    proj_v_sub(ic4, sub, strip, vset)

            def scores_block(pp, qc, kc, q_ap, pt_out, vs, kT):
                """Scores for both heads of pair pp on k chunk kc, exp'd
                into pt_out[:, :, vs:512) ([128, 2, 512] view)."""
                ksl = slice(kc * KCH, (kc + 1) * KCH)
                sp = pssc.tile([P, 2, QCH], F32, tag="sc", name="smm")
                for hh in range(2):
                    base = hh * D
                    nc.tensor.matmul(
                        sp[:, hh, vs:],
                        lhsT=kT[pp][base:base + D, ksl],
                        rhs=q_ap[base:base + D, vs:],
                        start=True, stop=True,
                        tile_position=(base, 0),
                    )
                nc.scalar.activation(
                    pt_out[:, :, vs:], sp[:, :, vs:],
                    mybir.ActivationFunctionType.Exp,
                    scale=exp_scale)

            def attn_block(pp, qc, q_ap, ao_tile, aset=0, stepper=None):
                kT, vt = kTs[aset], vts[aset]
                """Attention for head pair pp over q chunk qc.

                q_ap: [128, 2, 512] fp8 (chunk 1 zeroed)
                ao_tile: [128, 512] bf16 output (normalized attn @ v)
                """
                kcs = [kc for kc in range(NKC) if plan[(qc, kc)][0] != "skip"]
                if not kcs:
                    return
                avt = psav.tile([P, QCH], F32, tag="av", name="avA")
                avt2 = psav.tile([P, QCH], F32, tag="av", name="avB")
                avs = (avt, avt2)
                for avi, kc in enumerate(kcs):
                    kind, var, vs, ms, me = plan[(qc, kc)]
                    pt = ppool.tile([P, 2, QCH], BF16, tag="p", name="p")
                    scores_block(pp, qc, kc, q_ap, pt, vs, kT)
                    if kind == "partial" and me > ms:
                        # DVE, not GpSimd: measured Pool tensor_mul is ~2x
                        # slower than DVE and this hop sits between exp and
                        # the AV matmul on the critical path
                        for hh in range(2):
                            nc.vector.tensor_mul(
                                pt[:, hh, ms:me], pt[:, hh, ms:me],
                                mskt[:, var + ms:var + me])
                    for hh in range(2):
                        nc.tensor.matmul(
                            avs[hh][:, vs:],
                            lhsT=vt[2 * pp + hh][:, kc, :],
                            rhs=pt[:, hh, vs:],
                            start=(avi == 0), stop=(avi == len(kcs) - 1),
                            skip_group_check=True,
                        )
                    if stepper is not None:
                        stepper()
                # rows 64:128 of avs hold the denominator broadcast across
                # 64 partitions (from the pad-ones columns of vt)
                for hh in range(2):
                    rec = work.tile([D, QCH], F32, tag="rec", name="rec")
                    nc.vector.reciprocal(rec, avs[hh][D:2 * D, :])
                    nc.vector.tensor_mul(ao_tile[hh * D:(hh + 1) * D, :],
                                         avs[hh][0:D, :], rec)

            def outproj_one(ic, oc, ao_tiles):
                isl = slice((ic % (ICH // P)) * P, (ic % (ICH // P)) * P + P)
                osl = slice(oc * 512, (oc + 1) * 512)
                ps = psmm.tile([P, 512], F32, tag="mm", name="yps")
                for pp in range(PAIRS):
                    nc.tensor.matmul(
                        ps,
                        lhsT=ao_tiles[pp][:, isl],
                        rhs=wo[:, pp, osl],
                        start=(pp == 0), stop=(pp == PAIRS - 1),
                    )
                yt = work.tile([P, 512], F32, tag="yout", name="yt")
                # drain on the Act engine (Copy shares the exp table set) so
                # the outproj chain doesn't queue behind DVE rope/norm work
                nc.scalar.activation(yt, ps, mybir.ActivationFunctionType.Copy)
                nc.sync.dma_start(
                    out=y[ic * P:(ic + 1) * P, osl], in_=yt)

            def outproj(ic, ao_tiles):
                for oc in range(HID // 512):
                    outproj_one(ic, oc, ao_tiles)

            def outproj_strip(ic, ao_tiles):
                for sub in range(ICH // P):
                    outproj(ic * (ICH // P) + sub, ao_tiles)

            if streaming:
                nstrips = N // ICH if MAX_STRIPS is None else MAX_STRIPS
                pending = None               # (ic, ao_tiles) awaiting outproj

                def alloc_q():
                    return [qpool.tile([P, QCH], BF16, tag=f"qs{mc}",
                                       name=f"qs{mc}")
                            for mc in range(PAIRS)]

                # prologue: project strip 0
                proj_v_strip(0, strip=strip0_v, vset=0)
                q_cur = alloc_q()
                proj_qk_strip(xqT, wq, 0, lambda mc: q_cur[mc],
                              strip=strip0_q)
                proj_qk_strip(xkT, wk, 0,
                              lambda mc: kTs[0][mc][:, 0:ICH],
                              strip=strip0_k)
                # steady state: attention for strip ic interleaved with the
                # projections of the next strip, so the Act engine (exp, the
                # per-strip bottleneck) never drains
                total = REPEAT * nstrips
                for it in range(total):
                    ic = it % nstrips
                    cset = (it // nstrips) % 2
                    nic = (it + 1) % nstrips if it + 1 < total else None
                    nset = ((it + 1) // nstrips) % 2
                    ao_tiles = [aopool.tile([P, QCH], BF16,
                                            tag=f"aos{pp}", name=f"aos{pp}")
                                for pp in range(PAIRS)]
                    q_next = None
                    # prefetch next strip's x loads at the top of the
                    # iteration so the DMAs run during the attention pairs
                    # instead of just-in-time before the projections
                    if nic is not None:
                        ns_v = load_strip(xvT, nic, "pf_v", split=2)
                        ns_q = load_strip(xqT, nic, "pf_q", split=2)
                        ns_k = load_strip(xkT, nic, "pf_k", split=2)
                        q_next = alloc_q()
                    # filler chains: out-projection of the previous strip and
                    # projections of the next strip, round-robin interleaved
                    # between attention score/AV blocks so the PE always has
                    # ready work while the Act engine paces the exp stream
                    phase_lists = []
                    if pending is not None:
                        pic, paos = pending
                        phase_lists.append([
                            (lambda s=sub, o=oc: outproj_one(
                                pic * (ICH // P) + s, o, paos))
                            for sub in range(ICH // P)
                            for oc in range(HID // 512)])
                    if nic is not None:
                        qn = q_next
                        phase_lists.append([
                            (lambda s=sub: proj_v_sub(nic, s, ns_v, nset))
                            for sub in range(ICH // P)])
                        phase_lists.append([
                            (lambda m=mc: proj_qk_mc(
                                wq, nic, m, lambda mm: qn[mm], ns_q))
                            for mc in range(PAIRS)])
                        phase_lists.append([
                            (lambda m=mc: proj_qk_mc(
                                wk, nic, m,
                                lambda mm: kTs[nset][mm][
                                    :, nic * ICH:(nic + 1) * ICH], ns_k))
                            for mc in range(PAIRS)])
                    chains = [t for tup in zip_longest(*phase_lists)
                              for t in tup if t is not None]
                    nblk = 4 * len([kc for kc in range(NKC)
                                    if plan[(ic, kc)][0] != "skip"])
                    state = [0, 0]  # blocks seen, chains emitted

                    def stepper():
                        state[0] += 1
                        tgt = len(chains) * state[0] // max(nblk, 1)
                        while state[1] < tgt:
                            chains[state[1]]()
                            state[1] += 1

                    for pp in range(PAIRS):
                        attn_block(pp, ic, q_cur[pp], ao_tiles[pp], cset,
                                   stepper=stepper)
                    while state[1] < len(chains):
                        chains[state[1]]()
                        state[1] += 1
                    pending = (ic, ao_tiles)
                    q_cur = q_next
                outproj_strip(*pending)
            else:
                for _rep in range(REPEAT):
                    for ic in range(N // ICH):
                        pre = _rep == 0 and ic == 0
                        proj_v_strip(ic, strip=strip0_v if pre else None)
                        proj_qk_strip(
                            xqT, wq, ic,
                            lambda mc: qT[mc][:, ic * ICH:(ic + 1) * ICH],
                            strip=strip0_q if pre else None)
                        proj_qk_strip(
                            xkT, wk, ic,
                            lambda mc: kTs[0][mc][:, ic * ICH:(ic + 1) * ICH],
                            strip=strip0_k if pre else None)
                    for qc in range(NQC):
                        ao_tiles = [aopool.tile([P, QCH], BF16,
                                                tag=f"aos{pp}", name=f"aos{pp}")
                                    for pp in range(PAIRS)]
                        for pp in range(PAIRS):
                            attn_block(pp, qc,
                                       qT[pp][:, qc * QCH:(qc + 1) * QCH],
                                       ao_tiles[pp])
                        outproj_strip(qc, ao_tiles)

    nc.compile()
    return nc, masks_np


def _get_nc(is_causal, start_pos):
    key = (bool(is_causal), int(start_pos), REPEAT, MAX_STRIPS)
    if key not in _NC_CACHE:
        _NC_CACHE[key] = _build_nc(bool(is_causal), int(start_pos))
    return _NC_CACHE[key]


# ---------------------------------------------------------------- entry
def kernel(x_q, x_k, x_v, W_q, W_k, W_v, W_out, padding_mask, is_causal,
           start_pos):
    x_q = np.asarray(x_q, dtype=np.float32)
    x_k = np.asarray(x_k, dtype=np.float32)
    x_v = np.asarray(x_v, dtype=np.float32)
    W_q = np.asarray(W_q, dtype=np.float32)
    W_k = np.asarray(W_k, dtype=np.float32)
    W_v = np.asarray(W_v, dtype=np.float32)
    W_out = np.asarray(W_out, dtype=np.float32)
    padding_mask = np.asarray(padding_mask).astype(bool)
    is_causal = int(np.asarray(is_causal))
    start_pos = int(np.asarray(start_pos))

    nc, masks = _get_nc(is_causal, start_pos)

    cos2, sin2 = _rope_tables()

    in_maps = []
    for c in range(NCORES):
        bi, hg = divmod(c, GROUPS)
        hs = hg * HD
        kpad = np.ascontiguousarray(
            padding_mask[bi].astype(np.float32).reshape(NKC, P).T
        ).astype(BF16NP)
        in_maps.append({
            "xqT": np.ascontiguousarray(x_q[bi].T).astype(BF16NP),
            "xkT": np.ascontiguousarray(x_k[bi].T).astype(BF16NP),
            "xvT": np.ascontiguousarray(x_v[bi].T).astype(BF16NP),
            "wqT": np.ascontiguousarray(W_q[hs:hs + HD].T).astype(BF16NP),
            "wkT": np.ascontiguousarray(W_k[hs:hs + HD].T).astype(BF16NP),
            "wvT": np.ascontiguousarray(W_v[hs:hs + HD].T).astype(BF16NP),
            "woT": np.ascontiguousarray(W_out[:, hs:hs + HD].T).astype(BF16NP),
            "cos": cos2.astype(BF16NP),
            "sin": sin2.astype(BF16NP),
            "masks": masks.astype(BF16NP),
            "kpad": kpad,
        })

    res = run_bass_kernel_spmd(nc, in_maps, list(range(NCORES)))
    out = np.empty((B, N, HID), dtype=np.float32)
    for bi in range(B):
        out[bi] = res.results[GROUPS * bi]["y"]
        for g in range(1, GROUPS):
            out[bi] += res.results[GROUPS * bi + g]["y"]
    return out



# revision 23
# speedup vs baseline: 1.0297x; 1.0297x over previous
"""Multi-head attention (RoPE, causal) Trainium2 Bass kernel.

Sharding (8 cores): data-parallel over batch (4) x tensor-parallel over
heads (16 -> 2 groups of 8).  Core c handles batch c//2 and head group
c%2.  Attention is fully head-local; the out-projection partial sums of
the two head groups of each batch are added on the host.

Per-core device kernel (sizes hardcoded for b=4, n=2048, hidden=1024,
h=16, d=64).  All matmuls in bf16 with fp32 PSUM accumulation (an fp8
DoubleRow variant was 2x faster on the PE but exceeded the 2e-2 error
budget; any single fp8 stage already costs >=4e-2 max-rel error):
  - QKV projections as bf16 matmuls; x and W arrive host-pre-transposed
    (c-major) and pre-cast, so no on-device transposes.
  - RoPE in head-transposed layout: u = q*sinP on DVE (sinP pair-swapped
    on the host so the +-1 pair-permutation matmul PM @ u on the PE
    directly yields rot(q)*sin), then q*cos + that on DVE; all DVE
    operands are bf16 SBUF so the 2x mode applies.
  - Scores computed transposed, s_T[k, q], two heads of a pair row-tiled
    onto the PE at tile_position (0,0)/(64,0), into one [128, 2, 512]
    PSUM tile so a single Exp activation (scale=1/8 folded in) covers
    both heads.  Diagonal blocks compute only the valid q column range
    [vs:512); the <=128-wide partial triangle is zeroed by a bf16 mask
    multiply on the otherwise-idle GpSimd engine.
  - AV: v augmented with 64 pad-ones columns (M=128) so PSUM rows
    64:128 hold the softmax denominator already broadcast across 64
    partitions; normalization is reciprocal+mul on DVE.
  - Software-pipelined strips: the projections of strip ic+1 are
    interleaved between the four attention pairs of strip ic so the Act
    engine (exp) never drains; the out-projection is deferred one strip
    so DVE normalization never stalls the PE.
"""

from itertools import zip_longest

import numpy as np
import ml_dtypes

import concourse.bass as bass
import concourse.mybir as mybir
from concourse import bacc
from concourse import hw_specs as _hw_specs
from concourse.tile import TileContext
from concourse.bass_utils import run_bass_kernel_spmd

# Calibrate the Tile scheduler's cost model to rates measured on this HW
# (microbenchmarks, steady-state chains): bf16 matmuls stream ~2 columns
# per cycle (F=512 chain ~150 ns vs the 213 ns 1-col/cycle default), the
# Act exp and DVE ops run ~0.7-0.8x the default cost, and GpSimd 2-input
# ops are ~2x SLOWER than the default.  A closer model gives the static
# per-engine schedule fewer runtime stalls.  Must run before the first
# compile in the process: the Rust cost model snapshots these class attrs
# into a process-global OnceCell on first use.
_hw_specs.TRN2Spec.PE_CYCLE = 1e9 / 3.2e9
_hw_specs.TRN2Spec.PE_CYCLE_PSTATE_MID = 1e9 / 1.6e9
_hw_specs.TRN2Spec.CYCLE_T = {
    **_hw_specs.TRN2Spec.CYCLE_T,
    mybir.EngineType.DVE: 0.60,
    mybir.EngineType.Activation: 0.625,
    mybir.EngineType.Pool: 1.67,
}

# ---------------------------------------------------------------- constants
B, N, HID = 4, 2048, 1024
H = 16
D = HID // H                     # 64
NCORES = 8
GROUPS = NCORES // B             # 2 head groups
HPG = H // GROUPS                # 8 heads per core
HD = HPG * D                     # 512 local head dims
PAIRS = HPG // 2                 # 4 head pairs per core
ROPE_THETA = 10000.0
SCALE = 0.125                    # 1/sqrt(d)
WSCALE = 1.0

P = 128
CC = HID // P                    # 8 contraction chunks for projections
ICH = 512                        # projection i-chunk (moving free dim)
QCH = 512                        # attention q-chunk
KCH = 128                        # attention k-chunk
NQC = N // QCH                   # 4
NKC = N // KCH                   # 16

F32 = mybir.dt.float32
BF16 = mybir.dt.bfloat16
FP8 = mybir.dt.float8e4
BF16NP = ml_dtypes.bfloat16
FP8NP = ml_dtypes.float8_e4m3
DR = mybir.MatmulPerfMode.DoubleRow

_NC_CACHE = {}
MAX_STRIPS = None
REPEAT = 1
ABLATE = frozenset()     # timing ablations: {"attn","proj","outproj","xdma"}


# ---------------------------------------------------------------- host prep
def _allow_matrix(is_causal, start_pos):
    i = np.arange(N)[:, None]    # query index
    j = np.arange(N)[None, :]    # key index
    if is_causal:
        return (j < start_pos) | ((i >= start_pos) & (i >= j))
    return np.ones((N, N), dtype=bool)


def _block_plan(is_causal, start_pos):
    """Classify each (qc, kc) score block.

    plan[(qc, kc)] is (kind, var, vs, ms, me):
      kind: 'skip' | 'full' | 'partial'
      var:  mask variant index (partial only)
      vs:   first valid q column in the 512-wide chunk (block computed
            over [vs:512) only)
      [ms, me): q column range where the partial triangle needs masking
    """
    allow = _allow_matrix(is_causal, start_pos)
    plan = {}
    variants = []
    vkeys = {}
    for qc in range(NQC):
        for kc in range(NKC):
            blk = allow[qc * QCH:(qc + 1) * QCH, kc * KCH:(kc + 1) * KCH]
            if not blk.any():
                plan[(qc, kc)] = ("skip", None, 0, 0, 0)
                continue
            if blk.all():
                plan[(qc, kc)] = ("full", None, 0, 0, 0)
                continue
            bT = blk.T               # [128 k, 512 q]
            start = np.argmax(bT, axis=1)
            for r in range(KCH):
                if not bT[r].any():
                    raise NotImplementedError("empty k-row in partial block")
                s = start[r]
                if not bT[r, s:].all() or bT[r, :s].any():
                    raise NotImplementedError("non-suffix mask row")
            key = start.tobytes()
            if key not in vkeys:
                vkeys[key] = len(variants)
                variants.append(start.astype(np.float32))
            vs = int(start.min())
            me = int(start.max())
            plan[(qc, kc)] = ("partial", vkeys[key], vs, vs, me)
    if not variants:
        variants.append(np.zeros(KCH, dtype=np.float32))
    # sanity: first non-skip block per qc row must cover the full q range
    for qc in range(NQC):
        for kc in range(NKC):
            kind, _, vs, _, _ = plan[(qc, kc)]
            if kind != "skip":
                assert vs == 0, f"first block of row {qc} has vs={vs}"
                break
    # If every variant is a slope-1 diagonal start[k] = k + o (the causal
    # case), all masks are column slices of ONE [128, QCH + o_max] ramp
    # R[k, t] = (t >= k + o_max): variant o lives at column offset
    # o_max - o.  That shrinks the SBUF table ~2.3x vs concatenation.
    offs = []
    for v in variants:
        o = int(v[0])
        if np.array_equal(v, np.arange(KCH) + o):
            offs.append(o)
        else:
            offs.append(None)
    if all(o is not None for o in offs) and variants:
        o_max = max(offs)
        k = np.arange(KCH)[:, None]
        t = np.arange(QCH + o_max)[None, :]
        masks = (t >= k + o_max).astype(np.float32)   # [128, QCH + o_max]
        for key in plan:
            kind, var, vs, ms, me = plan[key]
            if kind == "partial":
                plan[key] = (kind, o_max - offs[var], vs, ms, me)
    else:
        # fallback: concatenated per-variant blocks at offsets var*QCH
        q = np.arange(QCH)[None, :]
        blocks = [(q >= v[:, None]).astype(np.float32) for v in variants]
        masks = np.concatenate(blocks, axis=1)        # [128, V*QCH]
        for key in plan:
            kind, var, vs, ms, me = plan[key]
            if kind == "partial":
                plan[key] = (kind, var * QCH, vs, ms, me)
    return plan, masks


def _rope_tables():
    inv_freq = 1.0 / (ROPE_THETA ** (np.arange(0, D, 2, dtype=np.float64) / D))
    t = np.arange(N, dtype=np.float64)
    freqs = t[:, None] * inv_freq[None, :]        # [N, 32]
    freqs = np.repeat(freqs, 2, axis=1)           # [N, 64]
    cos = np.cos(freqs).T.astype(np.float32)      # [64, N]
    sin = np.sin(freqs).T.astype(np.float32)
    # sinN folds the rotate-half signs: dst = raw*cos + pairswap(raw)*sinN
    # with sinN[2r] = -sin[2r], sinN[2r+1] = +sin[2r+1]; the pair swap is a
    # partition-strided SBUF->SBUF DMA so no PE/PSUM is involved.
    sinN = sin.copy()
    sinN[0::2] = -sin[0::2]
    # duplicate rows so both heads of a pair (partitions 0:64 / 64:128)
    # see the same table at matching partition base
    cos2 = np.concatenate([cos, cos], axis=0)     # [128, N]
    sin2 = np.concatenate([sinN, sinN], axis=0)
    return np.ascontiguousarray(cos2), np.ascontiguousarray(sin2)


# ---------------------------------------------------------------- device IR
def _build_nc(is_causal, start_pos):
    plan, masks_np = _block_plan(is_causal, start_pos)
    mwid = masks_np.shape[1]
    streaming = bool(is_causal)
    exp_scale = SCALE

    nc = bacc.Bacc("TRN2", target_bir_lowering=False, debug=False)

    xqT = nc.declare_dram_parameter("xqT", [HID, N], BF16, isOutput=False).ap()
    xkT = nc.declare_dram_parameter("xkT", [HID, N], BF16, isOutput=False).ap()
    xvT = nc.declare_dram_parameter("xvT", [HID, N], BF16, isOutput=False).ap()
    wqT = nc.declare_dram_parameter("wqT", [HID, HD], BF16, isOutput=False).ap()
    wkT = nc.declare_dram_parameter("wkT", [HID, HD], BF16, isOutput=False).ap()
    wvT = nc.declare_dram_parameter("wvT", [HID, HD], BF16, isOutput=False).ap()
    woT = nc.declare_dram_parameter("woT", [HD, HID], BF16, isOutput=False).ap()
    cos_d = nc.declare_dram_parameter("cos", [P, N], BF16, isOutput=False).ap()
    sin_d = nc.declare_dram_parameter("sin", [P, N], BF16, isOutput=False).ap()
    msk_d = nc.declare_dram_parameter("masks", [P, mwid], BF16,
                                      isOutput=False).ap()
    kpad_d = nc.declare_dram_parameter("kpad", [P, NKC], BF16, isOutput=False).ap()
    y = nc.declare_dram_parameter("y", [N, HID], F32, isOutput=True).ap()

    with TileContext(nc) as tc:
        with (
            tc.tile_pool(name="const", bufs=1) as const,
            tc.tile_pool(name="persist", bufs=1) as persist,
            tc.tile_pool(name="xstrip", bufs=3) as xpool,
            tc.tile_pool(name="qpool", bufs=2) as qpool,
            tc.tile_pool(name="aopool", bufs=2) as aopool,
            tc.tile_pool(name="work", bufs=2) as work,
            tc.tile_pool(name="ppool", bufs=6) as ppool,
            tc.tile_pool(name="psmm", bufs=2, space="PSUM") as psmm,
            tc.tile_pool(name="pssc", bufs=2, space="PSUM") as pssc,
            tc.tile_pool(name="psav", bufs=2, space="PSUM") as psav,
        ):
            # ---------------- weights first: the first projections need them
            wv = const.tile([P, CC, HD], BF16, tag="wv", name="wv")
            nc.sync.dma_start(out=wv, in_=wvT.rearrange("(cc p) m -> p cc m", p=P))
            wq = const.tile([P, CC, HD], BF16, tag="wq", name="wq")
            nc.sync.dma_start(out=wq, in_=wqT.rearrange("(cc p) m -> p cc m", p=P))

            # strip-0 x loads, issued before the rest of the tables
            def load_strip(x_dram, ic, nm, split=1):
                strip = xpool.tile([P, CC, ICH], BF16, tag="xstrip", name=nm)
                src = x_dram.rearrange("(cc p) n -> p cc n", p=P)[
                    :, :, ic * ICH:(ic + 1) * ICH]
                step = CC // split
                if "xdma" not in ABLATE:
                    for s in range(split):
                        nc.sync.dma_start(
                            out=strip[:, s * step:(s + 1) * step],
                            in_=src[:, s * step:(s + 1) * step])
                return strip

            strip0_v = load_strip(xvT, 0, "strip0v")
            strip0_q = load_strip(xqT, 0, "strip0q")
            wk = const.tile([P, CC, HD], BF16, tag="wk", name="wk")
            nc.sync.dma_start(out=wk, in_=wkT.rearrange("(cc p) m -> p cc m", p=P))
            strip0_k = load_strip(xkT, 0, "strip0k")

            # ---------------- remaining constants / tables
            cost = const.tile([P, N], BF16, tag="cos", name="cos")
            sint = const.tile([P, N], BF16, tag="sin", name="sin")
            nc.sync.dma_start(out=cost, in_=cos_d)
            nc.sync.dma_start(out=sint, in_=sin_d)
            mskt = const.tile([P, mwid], BF16, tag="masks", name="mskt")
            nc.sync.dma_start(out=mskt, in_=msk_d)
            kpad = const.tile([P, NKC], BF16, tag="kpad", name="kpad")
            nc.sync.dma_start(out=kpad, in_=kpad_d)
            wo = const.tile([P, PAIRS, HID], BF16, tag="wo", name="wo")
            nc.sync.dma_start(out=wo, in_=woT.rearrange("(jc p) o -> p jc o", p=P))

            # ---------------- persistent activations
            # roped k per pair, double-buffered across repeats so the
            # next repeat's projections never wait on this one's attention
            kTs = [[persist.tile([P, N], BF16, tag=f"kT{s}{p}",
                                 name=f"kT{s}{p}")
                    for p in range(PAIRS)] for s in range(2)]
            kT = kTs[0]
            # v: per (head, k-chunk) blocks of [v(64) | pad-ones(64)]
            vtalls = [persist.tile([P, HPG, NKC, P], BF16, tag=f"vt{s}",
                                   name=f"vt{s}") for s in range(2)]
            vts = [[va[:, h] for h in range(HPG)] for va in vtalls]
            vtall, vt = vtalls[0], vts[0]
            if streaming:
                qT = None
            else:
                qT = [persist.tile([P, N], BF16, tag=f"qT{p}", name=f"qT{p}")
                      for p in range(PAIRS)]

            # pad-ones halves of vt, written once on the (idle) gpsimd engine
            for va in vtalls:
                for kc in range(NKC):
                    nc.gpsimd.tensor_copy(
                        out=va[:, :, kc, D:P],
                        in_=kpad[:, kc:kc + 1].to_broadcast([P, HPG, D]),
                    )

            def proj_qk_mc(w_sb, ic, mc, dst_of_mc, strip):
                """One q/k projection chain (pair mc of a 512-wide strip)."""
                if "proj" in ABLATE:
                    nc.vector.tensor_copy(out=dst_of_mc(mc),
                                          in_=cost[:, 0:ICH])
                    return
                isl = slice(ic * ICH, (ic + 1) * ICH)
                ps = psmm.tile([P, ICH], F32, tag="mm", name="pjmm")
                for cc in range(CC):
                    nc.tensor.matmul(
                        ps,
                        lhsT=w_sb[:, cc, mc * P:(mc + 1) * P],
                        rhs=strip[:, cc, :],
                        start=(cc == 0),
                        stop=(cc == CC - 1),
                    )
                raw = work.tile([P, ICH], BF16, tag="raw", name="raw")
                nc.vector.tensor_copy(out=raw, in_=ps)
                # rotate-half as a partition pair swap on the (idle) DMA
                # engines; the signs live in the sinN table
                rsw = work.tile([P, ICH], BF16, tag="rsw", name="rsw")
                nc.sync.dma_start(out=rsw[0:P - 1:2], in_=raw[1:P:2])
                nc.sync.dma_start(out=rsw[1:P:2], in_=raw[0:P - 1:2])
                u = work.tile([P, ICH], BF16, tag="ropeu", name="u")
                nc.vector.tensor_mul(u, rsw, sint[:, isl])
                cw = work.tile([P, ICH], BF16, tag="ropecw", name="cw")
                nc.vector.tensor_mul(cw, raw, cost[:, isl])
                nc.vector.tensor_add(dst_of_mc(mc), cw, u)

            def proj_qk_strip(x_dram, w_sb, ic, dst_of_mc, strip=None):
                """Project one 512-wide strip of q or k (all pairs) + RoPE."""
                if strip is None:
                    strip = load_strip(x_dram, ic, "strip")
                for mc in range(PAIRS):
                    proj_qk_mc(w_sb, ic, mc, dst_of_mc, strip)

            def proj_v_sub(ic4, sub, strip, vset):
                """Project one 128-wide sub-chunk of a v strip into vt."""
                if "proj" in ABLATE:
                    return
                vta = vtalls[vset]
                ic = ic4 * (ICH // P) + sub
                ps = psmm.tile([P, HD], F32, tag="mm", name="pvmm")
                for cc in range(CC):
                    nc.tensor.matmul(
                        ps,
                        lhsT=strip[:, cc, sub * P:(sub + 1) * P],
                        rhs=wv[:, cc, :],
                        start=(cc == 0),
                        stop=(cc == CC - 1),
                    )
                padb = kpad[:, ic:ic + 1]
                nc.vector.tensor_mul(
                    vta[:, :, ic, 0:D],
                    ps.rearrange("p (h d) -> p h d", h=HPG),
                    padb.to_broadcast([P, HPG, D]),
                )

            def proj_v_strip(ic4, strip=None, vset=0):
                """Project one 512-wide strip of v into vt (pad-scaled)."""
                if strip is None:
                    strip = load_strip(xvT, ic4, "vstrip")
                for sub in range(ICH // P):
                    proj_v_sub(ic4, sub, strip, vset)

            def scores_block(pp, qc, kc, q_ap, pt_out, vs, kT):
                """Scores for both heads of pair pp on k chunk kc, exp'd
                into pt_out[:, :, vs:512) ([128, 2, 512] view)."""
                ksl = slice(kc * KCH, (kc + 1) * KCH)
                sp = pssc.tile([P, 2, QCH], F32, tag="sc", name="smm")
                # high priority: the exp->scores round trip paces the whole
                # attention stream (the score-PSUM slot frees when the exp
                # two blocks back completes).  Without the boost the PE
                # stream runs the same-sem-gated AV matmuls first and the
                # Act engine starves for ~600 ns every block.
                with tc.high_priority(offset=64):
                    for hh in range(2):
                        base = hh * D
                        nc.tensor.matmul(
                            sp[:, hh, vs:],
                            lhsT=kT[pp][base:base + D, ksl],
                            rhs=q_ap[base:base + D, vs:],
                            start=True, stop=True,
                            tile_position=(base, 0),
                        )
                    nc.scalar.activation(
                        pt_out[:, :, vs:], sp[:, :, vs:],
                        mybir.ActivationFunctionType.Exp,
                        scale=exp_scale)

            def attn_block(pp, qc, q_ap, ao_tile, aset=0, stepper=None):
                kT, vt = kTs[aset], vts[aset]
                """Attention for head pair pp over q chunk qc.

                q_ap: [128, 2, 512] fp8 (chunk 1 zeroed)
                ao_tile: [128, 512] bf16 output (normalized attn @ v)
                """
                kcs = [kc for kc in range(NKC) if plan[(qc, kc)][0] != "skip"]
                if not kcs or "attn" in ABLATE:
                    if "attn" in ABLATE:
                        nc.vector.tensor_copy(out=ao_tile, in_=cost[:, 0:QCH])
                    if stepper is not None:
                        for _ in kcs:
                            stepper()
                    return
                avt = psav.tile([P, QCH], F32, tag="av", name="avA")
                avt2 = psav.tile([P, QCH], F32, tag="av", name="avB")
                avs = (avt, avt2)
                for avi, kc in enumerate(kcs):
                    kind, var, vs, ms, me = plan[(qc, kc)]
                    pt = ppool.tile([P, 2, QCH], BF16, tag="p", name="p")
                    scores_block(pp, qc, kc, q_ap, pt, vs, kT)
                    if kind == "partial" and me > ms:
                        # DVE, not GpSimd: measured Pool tensor_mul is ~2x
                        # slower than DVE and this hop sits between exp and
                        # the AV matmul on the critical path
                        with tc.high_priority(offset=32):
                            for hh in range(2):
                                nc.vector.tensor_mul(
                                    pt[:, hh, ms:me], pt[:, hh, ms:me],
                                    mskt[:, var + ms:var + me])
                    for hh in range(2):
                        nc.tensor.matmul(
                            avs[hh][:, vs:],
                            lhsT=vt[2 * pp + hh][:, kc, :],
                            rhs=pt[:, hh, vs:],
                            start=(avi == 0), stop=(avi == len(kcs) - 1),
                            skip_group_check=True,
                        )
                    if stepper is not None:
                        stepper()
                # rows 64:128 of avs hold the denominator broadcast across
                # 64 partitions (from the pad-ones columns of vt)
                for hh in range(2):
                    rec = work.tile([D, QCH], F32, tag="rec", name="rec")
                    nc.vector.reciprocal(rec, avs[hh][D:2 * D, :])
                    nc.vector.tensor_mul(ao_tile[hh * D:(hh + 1) * D, :],
                                         avs[hh][0:D, :], rec)

            def outproj_one(ic, oc, ao_tiles):
                if "outproj" in ABLATE:
                    return
                isl = slice((ic % (ICH // P)) * P, (ic % (ICH // P)) * P + P)
                osl = slice(oc * 512, (oc + 1) * 512)
                ps = psmm.tile([P, 512], F32, tag="mm", name="yps")
                for pp in range(PAIRS):
                    nc.tensor.matmul(
                        ps,
                        lhsT=ao_tiles[pp][:, isl],
                        rhs=wo[:, pp, osl],
                        start=(pp == 0), stop=(pp == PAIRS - 1),
                    )
                yt = work.tile([P, 512], F32, tag="yout", name="yt")
                # drain on the Act engine (Copy shares the exp table set) so
                # the outproj chain doesn't queue behind DVE rope/norm work
                nc.scalar.activation(yt, ps, mybir.ActivationFunctionType.Copy)
                nc.sync.dma_start(
                    out=y[ic * P:(ic + 1) * P, osl], in_=yt)

            def outproj(ic, ao_tiles):
                for oc in range(HID // 512):
                    outproj_one(ic, oc, ao_tiles)

            def outproj_strip(ic, ao_tiles):
                for sub in range(ICH // P):
                    outproj(ic * (ICH // P) + sub, ao_tiles)

            if streaming:
                nstrips = N // ICH if MAX_STRIPS is None else MAX_STRIPS
                pending = None               # (ic, ao_tiles) awaiting outproj

                def alloc_q():
                    return [qpool.tile([P, QCH], BF16, tag=f"qs{mc}",
                                       name=f"qs{mc}")
                            for mc in range(PAIRS)]

                # prologue: project strip 0
                proj_v_strip(0, strip=strip0_v, vset=0)
                q_cur = alloc_q()
                proj_qk_strip(xqT, wq, 0, lambda mc: q_cur[mc],
                              strip=strip0_q)
                proj_qk_strip(xkT, wk, 0,
                              lambda mc: kTs[0][mc][:, 0:ICH],
                              strip=strip0_k)
                # steady state: attention for strip ic interleaved with the
                # projections of the next strip, so the Act engine (exp, the
                # per-strip bottleneck) never drains
                total = REPEAT * nstrips
                for it in range(total):
                    ic = it % nstrips
                    cset = (it // nstrips) % 2
                    nic = (it + 1) % nstrips if it + 1 < total else None
                    nset = ((it + 1) // nstrips) % 2
                    ao_tiles = [aopool.tile([P, QCH], BF16,
                                            tag=f"aos{pp}", name=f"aos{pp}")
                                for pp in range(PAIRS)]
                    q_next = None
                    # prefetch next strip's x loads at the top of the
                    # iteration so the DMAs run during the attention pairs
                    # instead of just-in-time before the projections
                    if nic is not None:
                        ns_v = load_strip(xvT, nic, "pf_v", split=2)
                        ns_q = load_strip(xqT, nic, "pf_q", split=2)
                        ns_k = load_strip(xkT, nic, "pf_k", split=2)
                        q_next = alloc_q()
                    # filler chains: out-projection of the previous strip and
                    # projections of the next strip, round-robin interleaved
                    # between attention score/AV blocks so the PE always has
                    # ready work while the Act engine paces the exp stream
                    phase_lists = []
                    if pending is not None:
                        pic, paos = pending
                        phase_lists.append([
                            (lambda s=sub, o=oc: outproj_one(
                                pic * (ICH // P) + s, o, paos))
                            for sub in range(ICH // P)
                            for oc in range(HID // 512)])
                    if nic is not None:
                        qn = q_next
                        phase_lists.append([
                            (lambda s=sub: proj_v_sub(nic, s, ns_v, nset))
                            for sub in range(ICH // P)])
                        phase_lists.append([
                            (lambda m=mc: proj_qk_mc(
                                wq, nic, m, lambda mm: qn[mm], ns_q))
                            for mc in range(PAIRS)])
                        phase_lists.append([
                            (lambda m=mc: proj_qk_mc(
                                wk, nic, m,
                                lambda mm: kTs[nset][mm][
                                    :, nic * ICH:(nic + 1) * ICH], ns_k))
                            for mc in range(PAIRS)])
                    chains = [t for tup in zip_longest(*phase_lists)
                              for t in tup if t is not None]
                    nblk = 4 * len([kc for kc in range(NKC)
                                    if plan[(ic, kc)][0] != "skip"])
                    state = [0, 0]  # blocks seen, chains emitted

                    def stepper():
                        state[0] += 1
                        tgt = len(chains) * state[0] // max(nblk, 1)
                        while state[1] < tgt:
                            chains[state[1]]()
                            state[1] += 1

                    for pp in range(PAIRS):
                        attn_block(pp, ic, q_cur[pp], ao_tiles[pp], cset,
                                   stepper=stepper)
                    while state[1] < len(chains):
                        chains[state[1]]()
                        state[1] += 1
                    pending = (ic, ao_tiles)
                    q_cur = q_next
                outproj_strip(*pending)
            else:
                for _rep in range(REPEAT):
                    for ic in range(N // ICH):
                        pre = _rep == 0 and ic == 0
                        proj_v_strip(ic, strip=strip0_v if pre else None)
                        proj_qk_strip(
                            xqT, wq, ic,
                            lambda mc: qT[mc][:, ic * ICH:(ic + 1) * ICH],
                            strip=strip0_q if pre else None)
                        proj_qk_strip(
                            xkT, wk, ic,
                            lambda mc: kTs[0][mc][:, ic * ICH:(ic + 1) * ICH],
                            strip=strip0_k if pre else None)
                    for qc in range(NQC):
                        ao_tiles = [aopool.tile([P, QCH], BF16,
                                                tag=f"aos{pp}", name=f"aos{pp}")
                                    for pp in range(PAIRS)]
                        for pp in range(PAIRS):
                            attn_block(pp, qc,
                                       qT[pp][:, qc * QCH:(qc + 1) * QCH],
                                       ao_tiles[pp])
                        outproj_strip(qc, ao_tiles)

    nc.compile()
    return nc, masks_np


def _get_nc(is_causal, start_pos):
    key = (bool(is_causal), int(start_pos), REPEAT, MAX_STRIPS, ABLATE)
    if key not in _NC_CACHE:
        _NC_CACHE[key] = _build_nc(bool(is_causal), int(start_pos))
    return _NC_CACHE[key]


# ---------------------------------------------------------------- entry
def kernel(x_q, x_k, x_v, W_q, W_k, W_v, W_out, padding_mask, is_causal,
           start_pos):
    x_q = np.asarray(x_q, dtype=np.float32)
    x_k = np.asarray(x_k, dtype=np.float32)
    x_v = np.asarray(x_v, dtype=np.float32)
    W_q = np.asarray(W_q, dtype=np.float32)
    W_k = np.asarray(W_k, dtype=np.float32)
    W_v = np.asarray(W_v, dtype=np.float32)
    W_out = np.asarray(W_out, dtype=np.float32)
    padding_mask = np.asarray(padding_mask).astype(bool)
    is_causal = int(np.asarray(is_causal))
    start_pos = int(np.asarray(start_pos))

    nc, masks = _get_nc(is_causal, start_pos)

    cos2, sin2 = _rope_tables()

    in_maps = []
    for c in range(NCORES):
        bi, hg = divmod(c, GROUPS)
        hs = hg * HD
        kpad = np.ascontiguousarray(
            padding_mask[bi].astype(np.float32).reshape(NKC, P).T
        ).astype(BF16NP)
        in_maps.append({
            "xqT": np.ascontiguousarray(x_q[bi].T).astype(BF16NP),
            "xkT": np.ascontiguousarray(x_k[bi].T).astype(BF16NP),
            "xvT": np.ascontiguousarray(x_v[bi].T).astype(BF16NP),
            "wqT": np.ascontiguousarray(W_q[hs:hs + HD].T).astype(BF16NP),
            "wkT": np.ascontiguousarray(W_k[hs:hs + HD].T).astype(BF16NP),
            "wvT": np.ascontiguousarray(W_v[hs:hs + HD].T).astype(BF16NP),
            "woT": np.ascontiguousarray(W_out[:, hs:hs + HD].T).astype(BF16NP),
            "cos": cos2.astype(BF16NP),
            "sin": sin2.astype(BF16NP),
            "masks": masks.astype(BF16NP),
            "kpad": kpad,
        })

    res = run_bass_kernel_spmd(nc, in_maps, list(range(NCORES)))
    out = np.empty((B, N, HID), dtype=np.float32)
    for bi in range(B):
        out[bi] = res.results[GROUPS * bi]["y"]
        for g in range(1, GROUPS):
            out[bi] += res.results[GROUPS * bi + g]["y"]
    return out



# revision 24
# speedup vs baseline: 1.0581x; 1.0275x over previous
"""Multi-head attention (RoPE, causal) Trainium2 Bass kernel.

Sharding (8 cores): data-parallel over batch (4) x tensor-parallel over
heads (16 -> 2 groups of 8).  Core c handles batch c//2 and head group
c%2.  Attention is fully head-local; the out-projection partial sums of
the two head groups of each batch are added on the host.

Per-core device kernel (sizes hardcoded for b=4, n=2048, hidden=1024,
h=16, d=64).  All matmuls in bf16 with fp32 PSUM accumulation (an fp8
DoubleRow variant was 2x faster on the PE but exceeded the 2e-2 error
budget; any single fp8 stage already costs >=4e-2 max-rel error):
  - QKV projections as bf16 matmuls; x and W arrive host-pre-transposed
    (c-major) and pre-cast, so no on-device transposes.
  - RoPE in head-transposed layout: u = q*sinP on DVE (sinP pair-swapped
    on the host so the +-1 pair-permutation matmul PM @ u on the PE
    directly yields rot(q)*sin), then q*cos + that on DVE; all DVE
    operands are bf16 SBUF so the 2x mode applies.
  - Scores computed transposed, s_T[k, q], two heads of a pair row-tiled
    onto the PE at tile_position (0,0)/(64,0), into one [128, 2, 512]
    PSUM tile so a single Exp activation (scale=1/8 folded in) covers
    both heads.  Diagonal blocks compute only the valid q column range
    [vs:512); the <=128-wide partial triangle is zeroed by a bf16 mask
    multiply on the otherwise-idle GpSimd engine.
  - AV: v augmented with 64 pad-ones columns (M=128) so PSUM rows
    64:128 hold the softmax denominator already broadcast across 64
    partitions; normalization is reciprocal+mul on DVE.
  - Software-pipelined strips: the projections of strip ic+1 are
    interleaved between the four attention pairs of strip ic so the Act
    engine (exp) never drains; the out-projection is deferred one strip
    so DVE normalization never stalls the PE.
"""

from itertools import zip_longest

import numpy as np
import ml_dtypes

import concourse.bass as bass
import concourse.mybir as mybir
from concourse import bacc
from concourse import hw_specs as _hw_specs
from concourse.tile import TileContext
from concourse.bass_utils import run_bass_kernel_spmd

# Calibrate the Tile scheduler's cost model to rates measured on this HW
# (microbenchmarks, steady-state chains): bf16 matmuls stream ~2 columns
# per cycle (F=512 chain ~150 ns vs the 213 ns 1-col/cycle default), the
# Act exp and DVE ops run ~0.7-0.8x the default cost, and GpSimd 2-input
# ops are ~2x SLOWER than the default.  A closer model gives the static
# per-engine schedule fewer runtime stalls.  Must run before the first
# compile in the process: the Rust cost model snapshots these class attrs
# into a process-global OnceCell on first use.
_hw_specs.TRN2Spec.PE_CYCLE = 1e9 / 3.2e9
_hw_specs.TRN2Spec.PE_CYCLE_PSTATE_MID = 1e9 / 1.6e9
_hw_specs.TRN2Spec.CYCLE_T = {
    **_hw_specs.TRN2Spec.CYCLE_T,
    mybir.EngineType.DVE: 0.70,
    mybir.EngineType.Activation: 0.90,
    mybir.EngineType.Pool: 1.67,
}

# ---------------------------------------------------------------- constants
B, N, HID = 4, 2048, 1024
H = 16
D = HID // H                     # 64
NCORES = 8
GROUPS = NCORES // B             # 2 head groups
HPG = H // GROUPS                # 8 heads per core
HD = HPG * D                     # 512 local head dims
PAIRS = HPG // 2                 # 4 head pairs per core
ROPE_THETA = 10000.0
SCALE = 0.125                    # 1/sqrt(d)
WSCALE = 1.0

P = 128
CC = HID // P                    # 8 contraction chunks for projections
ICH = 512                        # projection i-chunk (moving free dim)
QCH = 512                        # attention q-chunk
KCH = 128                        # attention k-chunk
NQC = N // QCH                   # 4
NKC = N // KCH                   # 16

F32 = mybir.dt.float32
BF16 = mybir.dt.bfloat16
FP8 = mybir.dt.float8e4
BF16NP = ml_dtypes.bfloat16
FP8NP = ml_dtypes.float8_e4m3
DR = mybir.MatmulPerfMode.DoubleRow

_NC_CACHE = {}
MAX_STRIPS = None
REPEAT = 1
ABLATE = frozenset()     # timing ablations: {"attn","proj","outproj","xdma"}


# ---------------------------------------------------------------- host prep
def _allow_matrix(is_causal, start_pos):
    i = np.arange(N)[:, None]    # query index
    j = np.arange(N)[None, :]    # key index
    if is_causal:
        return (j < start_pos) | ((i >= start_pos) & (i >= j))
    return np.ones((N, N), dtype=bool)


def _block_plan(is_causal, start_pos):
    """Classify each (qc, kc) score block.

    plan[(qc, kc)] is (kind, var, vs, ms, me):
      kind: 'skip' | 'full' | 'partial'
      var:  mask variant index (partial only)
      vs:   first valid q column in the 512-wide chunk (block computed
            over [vs:512) only)
      [ms, me): q column range where the partial triangle needs masking
    """
    allow = _allow_matrix(is_causal, start_pos)
    plan = {}
    variants = []
    vkeys = {}
    for qc in range(NQC):
        for kc in range(NKC):
            blk = allow[qc * QCH:(qc + 1) * QCH, kc * KCH:(kc + 1) * KCH]
            if not blk.any():
                plan[(qc, kc)] = ("skip", None, 0, 0, 0)
                continue
            if blk.all():
                plan[(qc, kc)] = ("full", None, 0, 0, 0)
                continue
            bT = blk.T               # [128 k, 512 q]
            start = np.argmax(bT, axis=1)
            for r in range(KCH):
                if not bT[r].any():
                    raise NotImplementedError("empty k-row in partial block")
                s = start[r]
                if not bT[r, s:].all() or bT[r, :s].any():
                    raise NotImplementedError("non-suffix mask row")
            key = start.tobytes()
            if key not in vkeys:
                vkeys[key] = len(variants)
                variants.append(start.astype(np.float32))
            vs = int(start.min())
            me = int(start.max())
            plan[(qc, kc)] = ("partial", vkeys[key], vs, vs, me)
    if not variants:
        variants.append(np.zeros(KCH, dtype=np.float32))
    # sanity: first non-skip block per qc row must cover the full q range
    for qc in range(NQC):
        for kc in range(NKC):
            kind, _, vs, _, _ = plan[(qc, kc)]
            if kind != "skip":
                assert vs == 0, f"first block of row {qc} has vs={vs}"
                break
    # If every variant is a slope-1 diagonal start[k] = k + o (the causal
    # case), all masks are column slices of ONE [128, QCH + o_max] ramp
    # R[k, t] = (t >= k + o_max): variant o lives at column offset
    # o_max - o.  That shrinks the SBUF table ~2.3x vs concatenation.
    offs = []
    for v in variants:
        o = int(v[0])
        if np.array_equal(v, np.arange(KCH) + o):
            offs.append(o)
        else:
            offs.append(None)
    if all(o is not None for o in offs) and variants:
        o_max = max(offs)
        k = np.arange(KCH)[:, None]
        t = np.arange(QCH + o_max)[None, :]
        masks = (t >= k + o_max).astype(np.float32)   # [128, QCH + o_max]
        for key in plan:
            kind, var, vs, ms, me = plan[key]
            if kind == "partial":
                plan[key] = (kind, o_max - offs[var], vs, ms, me)
    else:
        # fallback: concatenated per-variant blocks at offsets var*QCH
        q = np.arange(QCH)[None, :]
        blocks = [(q >= v[:, None]).astype(np.float32) for v in variants]
        masks = np.concatenate(blocks, axis=1)        # [128, V*QCH]
        for key in plan:
            kind, var, vs, ms, me = plan[key]
            if kind == "partial":
                plan[key] = (kind, var * QCH, vs, ms, me)
    return plan, masks


def _rope_tables():
    inv_freq = 1.0 / (ROPE_THETA ** (np.arange(0, D, 2, dtype=np.float64) / D))
    t = np.arange(N, dtype=np.float64)
    freqs = t[:, None] * inv_freq[None, :]        # [N, 32]
    freqs = np.repeat(freqs, 2, axis=1)           # [N, 64]
    cos = np.cos(freqs).T.astype(np.float32)      # [64, N]
    sin = np.sin(freqs).T.astype(np.float32)
    # sinN folds the rotate-half signs: dst = raw*cos + pairswap(raw)*sinN
    # with sinN[2r] = -sin[2r], sinN[2r+1] = +sin[2r+1]; the pair swap is a
    # partition-strided SBUF->SBUF DMA so no PE/PSUM is involved.
    sinN = sin.copy()
    sinN[0::2] = -sin[0::2]
    # duplicate rows so both heads of a pair (partitions 0:64 / 64:128)
    # see the same table at matching partition base
    cos2 = np.concatenate([cos, cos], axis=0)     # [128, N]
    sin2 = np.concatenate([sinN, sinN], axis=0)
    return np.ascontiguousarray(cos2), np.ascontiguousarray(sin2)


# ---------------------------------------------------------------- device IR
def _build_nc(is_causal, start_pos):
    plan, masks_np = _block_plan(is_causal, start_pos)
    mwid = masks_np.shape[1]
    streaming = bool(is_causal)
    exp_scale = SCALE

    nc = bacc.Bacc("TRN2", target_bir_lowering=False, debug=False)

    xqT = nc.declare_dram_parameter("xqT", [HID, N], BF16, isOutput=False).ap()
    xkT = nc.declare_dram_parameter("xkT", [HID, N], BF16, isOutput=False).ap()
    xvT = nc.declare_dram_parameter("xvT", [HID, N], BF16, isOutput=False).ap()
    wqT = nc.declare_dram_parameter("wqT", [HID, HD], BF16, isOutput=False).ap()
    wkT = nc.declare_dram_parameter("wkT", [HID, HD], BF16, isOutput=False).ap()
    wvT = nc.declare_dram_parameter("wvT", [HID, HD], BF16, isOutput=False).ap()
    woT = nc.declare_dram_parameter("woT", [HD, HID], BF16, isOutput=False).ap()
    cos_d = nc.declare_dram_parameter("cos", [P, N], BF16, isOutput=False).ap()
    sin_d = nc.declare_dram_parameter("sin", [P, N], BF16, isOutput=False).ap()
    msk_d = nc.declare_dram_parameter("masks", [P, mwid], BF16,
                                      isOutput=False).ap()
    kpad_d = nc.declare_dram_parameter("kpad", [P, NKC], BF16, isOutput=False).ap()
    y = nc.declare_dram_parameter("y", [N, HID], F32, isOutput=True).ap()

    with TileContext(nc) as tc:
        with (
            tc.tile_pool(name="const", bufs=1) as const,
            tc.tile_pool(name="persist", bufs=1) as persist,
            tc.tile_pool(name="xstrip", bufs=3) as xpool,
            tc.tile_pool(name="qpool", bufs=2) as qpool,
            tc.tile_pool(name="aopool", bufs=2) as aopool,
            tc.tile_pool(name="work", bufs=2) as work,
            tc.tile_pool(name="ppool", bufs=6) as ppool,
            tc.tile_pool(name="psmm", bufs=2, space="PSUM") as psmm,
            tc.tile_pool(name="pssc", bufs=2, space="PSUM") as pssc,
            tc.tile_pool(name="psav", bufs=2, space="PSUM") as psav,
        ):
            # ---------------- weights first: the first projections need them
            wv = const.tile([P, CC, HD], BF16, tag="wv", name="wv")
            nc.sync.dma_start(out=wv, in_=wvT.rearrange("(cc p) m -> p cc m", p=P))
            wq = const.tile([P, CC, HD], BF16, tag="wq", name="wq")
            nc.sync.dma_start(out=wq, in_=wqT.rearrange("(cc p) m -> p cc m", p=P))

            # strip-0 x loads, issued before the rest of the tables
            def load_strip(x_dram, ic, nm, split=1):
                strip = xpool.tile([P, CC, ICH], BF16, tag="xstrip", name=nm)
                src = x_dram.rearrange("(cc p) n -> p cc n", p=P)[
                    :, :, ic * ICH:(ic + 1) * ICH]
                step = CC // split
                if "xdma" not in ABLATE:
                    for s in range(split):
                        nc.sync.dma_start(
                            out=strip[:, s * step:(s + 1) * step],
                            in_=src[:, s * step:(s + 1) * step])
                return strip

            strip0_v = load_strip(xvT, 0, "strip0v")
            strip0_q = load_strip(xqT, 0, "strip0q")
            wk = const.tile([P, CC, HD], BF16, tag="wk", name="wk")
            nc.sync.dma_start(out=wk, in_=wkT.rearrange("(cc p) m -> p cc m", p=P))
            strip0_k = load_strip(xkT, 0, "strip0k")

            # ---------------- remaining constants / tables
            cost = const.tile([P, N], BF16, tag="cos", name="cos")
            sint = const.tile([P, N], BF16, tag="sin", name="sin")
            nc.sync.dma_start(out=cost, in_=cos_d)
            nc.sync.dma_start(out=sint, in_=sin_d)
            mskt = const.tile([P, mwid], BF16, tag="masks", name="mskt")
            nc.sync.dma_start(out=mskt, in_=msk_d)
            kpad = const.tile([P, NKC], BF16, tag="kpad", name="kpad")
            nc.sync.dma_start(out=kpad, in_=kpad_d)
            wo = const.tile([P, PAIRS, HID], BF16, tag="wo", name="wo")
            nc.sync.dma_start(out=wo, in_=woT.rearrange("(jc p) o -> p jc o", p=P))

            # ---------------- persistent activations
            # roped k per pair, double-buffered across repeats so the
            # next repeat's projections never wait on this one's attention
            kTs = [[persist.tile([P, N], BF16, tag=f"kT{s}{p}",
                                 name=f"kT{s}{p}")
                    for p in range(PAIRS)] for s in range(2)]
            kT = kTs[0]
            # v: per (head, k-chunk) blocks of [v(64) | pad-ones(64)]
            vtalls = [persist.tile([P, HPG, NKC, P], BF16, tag=f"vt{s}",
                                   name=f"vt{s}") for s in range(2)]
            vts = [[va[:, h] for h in range(HPG)] for va in vtalls]
            vtall, vt = vtalls[0], vts[0]
            if streaming:
                qT = None
            else:
                qT = [persist.tile([P, N], BF16, tag=f"qT{p}", name=f"qT{p}")
                      for p in range(PAIRS)]

            # pad-ones halves of vt, written once on the (idle) gpsimd engine
            for va in vtalls:
                for kc in range(NKC):
                    nc.gpsimd.tensor_copy(
                        out=va[:, :, kc, D:P],
                        in_=kpad[:, kc:kc + 1].to_broadcast([P, HPG, D]),
                    )

            def proj_qk_mc(w_sb, ic, mc, dst_of_mc, strip):
                """One q/k projection chain (pair mc of a 512-wide strip)."""
                if "proj" in ABLATE:
                    nc.vector.tensor_copy(out=dst_of_mc(mc),
                                          in_=cost[:, 0:ICH])
                    return
                isl = slice(ic * ICH, (ic + 1) * ICH)
                ps = psmm.tile([P, ICH], F32, tag="mm", name="pjmm")
                for cc in range(CC):
                    nc.tensor.matmul(
                        ps,
                        lhsT=w_sb[:, cc, mc * P:(mc + 1) * P],
                        rhs=strip[:, cc, :],
                        start=(cc == 0),
                        stop=(cc == CC - 1),
                    )
                raw = work.tile([P, ICH], BF16, tag="raw", name="raw")
                nc.vector.tensor_copy(out=raw, in_=ps)
                # rotate-half as a partition pair swap on the (idle) DMA
                # engines; the signs live in the sinN table
                rsw = work.tile([P, ICH], BF16, tag="rsw", name="rsw")
                nc.sync.dma_start(out=rsw[0:P - 1:2], in_=raw[1:P:2])
                nc.sync.dma_start(out=rsw[1:P:2], in_=raw[0:P - 1:2])
                u = work.tile([P, ICH], BF16, tag="ropeu", name="u")
                nc.vector.tensor_mul(u, rsw, sint[:, isl])
                cw = work.tile([P, ICH], BF16, tag="ropecw", name="cw")
                nc.vector.tensor_mul(cw, raw, cost[:, isl])
                nc.vector.tensor_add(dst_of_mc(mc), cw, u)

            def proj_qk_strip(x_dram, w_sb, ic, dst_of_mc, strip=None):
                """Project one 512-wide strip of q or k (all pairs) + RoPE."""
                if strip is None:
                    strip = load_strip(x_dram, ic, "strip")
                for mc in range(PAIRS):
                    proj_qk_mc(w_sb, ic, mc, dst_of_mc, strip)

            def proj_v_sub(ic4, sub, strip, vset):
                """Project one 128-wide sub-chunk of a v strip into vt."""
                if "proj" in ABLATE:
                    return
                vta = vtalls[vset]
                ic = ic4 * (ICH // P) + sub
                ps = psmm.tile([P, HD], F32, tag="mm", name="pvmm")
                for cc in range(CC):
                    nc.tensor.matmul(
                        ps,
                        lhsT=strip[:, cc, sub * P:(sub + 1) * P],
                        rhs=wv[:, cc, :],
                        start=(cc == 0),
                        stop=(cc == CC - 1),
                    )
                padb = kpad[:, ic:ic + 1]
                nc.vector.tensor_mul(
                    vta[:, :, ic, 0:D],
                    ps.rearrange("p (h d) -> p h d", h=HPG),
                    padb.to_broadcast([P, HPG, D]),
                )

            def proj_v_strip(ic4, strip=None, vset=0):
                """Project one 512-wide strip of v into vt (pad-scaled)."""
                if strip is None:
                    strip = load_strip(xvT, ic4, "vstrip")
                for sub in range(ICH // P):
                    proj_v_sub(ic4, sub, strip, vset)

            def scores_block(pp, qc, kc, q_ap, pt_out, vs, kT):
                """Scores for both heads of pair pp on k chunk kc, exp'd
                into pt_out[:, :, vs:512) ([128, 2, 512] view)."""
                ksl = slice(kc * KCH, (kc + 1) * KCH)
                sp = pssc.tile([P, 2, QCH], F32, tag="sc", name="smm")
                # high priority: the exp->scores round trip paces the whole
                # attention stream (the score-PSUM slot frees when the exp
                # two blocks back completes).  Without the boost the PE
                # stream runs the same-sem-gated AV matmuls first and the
                # Act engine starves for ~600 ns every block.
                with tc.high_priority(offset=64):
                    for hh in range(2):
                        base = hh * D
                        nc.tensor.matmul(
                            sp[:, hh, vs:],
                            lhsT=kT[pp][base:base + D, ksl],
                            rhs=q_ap[base:base + D, vs:],
                            start=True, stop=True,
                            tile_position=(base, 0),
                        )
                    nc.scalar.activation(
                        pt_out[:, :, vs:], sp[:, :, vs:],
                        mybir.ActivationFunctionType.Exp,
                        scale=exp_scale)

            def attn_block(pp, qc, q_ap, ao_tile, aset=0, stepper=None):
                kT, vt = kTs[aset], vts[aset]
                """Attention for head pair pp over q chunk qc.

                q_ap: [128, 2, 512] fp8 (chunk 1 zeroed)
                ao_tile: [128, 512] bf16 output (normalized attn @ v)
                """
                kcs = [kc for kc in range(NKC) if plan[(qc, kc)][0] != "skip"]
                if not kcs or "attn" in ABLATE:
                    if "attn" in ABLATE:
                        nc.vector.tensor_copy(out=ao_tile, in_=cost[:, 0:QCH])
                    if stepper is not None:
                        for _ in kcs:
                            stepper()
                    return
                avt = psav.tile([P, QCH], F32, tag="av", name="avA")
                avt2 = psav.tile([P, QCH], F32, tag="av", name="avB")
                avs = (avt, avt2)
                for avi, kc in enumerate(kcs):
                    kind, var, vs, ms, me = plan[(qc, kc)]
                    pt = ppool.tile([P, 2, QCH], BF16, tag="p", name="p")
                    scores_block(pp, qc, kc, q_ap, pt, vs, kT)
                    if kind == "partial" and me > ms:
                        # DVE, not GpSimd: measured Pool tensor_mul is ~2x
                        # slower than DVE and this hop sits between exp and
                        # the AV matmul on the critical path
                        with tc.high_priority(offset=32):
                            for hh in range(2):
                                nc.vector.tensor_mul(
                                    pt[:, hh, ms:me], pt[:, hh, ms:me],
                                    mskt[:, var + ms:var + me])
                    for hh in range(2):
                        nc.tensor.matmul(
                            avs[hh][:, vs:],
                            lhsT=vt[2 * pp + hh][:, kc, :],
                            rhs=pt[:, hh, vs:],
                            start=(avi == 0), stop=(avi == len(kcs) - 1),
                            skip_group_check=True,
                        )
                    if stepper is not None:
                        stepper()
                # rows 64:128 of avs hold the denominator broadcast across
                # 64 partitions (from the pad-ones columns of vt)
                for hh in range(2):
                    rec = work.tile([D, QCH], F32, tag="rec", name="rec")
                    nc.vector.reciprocal(rec, avs[hh][D:2 * D, :])
                    nc.vector.tensor_mul(ao_tile[hh * D:(hh + 1) * D, :],
                                         avs[hh][0:D, :], rec)

            def outproj_one(ic, oc, ao_tiles):
                if "outproj" in ABLATE:
                    return
                isl = slice((ic % (ICH // P)) * P, (ic % (ICH // P)) * P + P)
                osl = slice(oc * 512, (oc + 1) * 512)
                ps = psmm.tile([P, 512], F32, tag="mm", name="yps")
                for pp in range(PAIRS):
                    nc.tensor.matmul(
                        ps,
                        lhsT=ao_tiles[pp][:, isl],
                        rhs=wo[:, pp, osl],
                        start=(pp == 0), stop=(pp == PAIRS - 1),
                    )
                yt = work.tile([P, 512], F32, tag="yout", name="yt")
                # drain on the Act engine (Copy shares the exp table set) so
                # the outproj chain doesn't queue behind DVE rope/norm work
                nc.scalar.activation(yt, ps, mybir.ActivationFunctionType.Copy)
                nc.sync.dma_start(
                    out=y[ic * P:(ic + 1) * P, osl], in_=yt)

            def outproj(ic, ao_tiles):
                for oc in range(HID // 512):
                    outproj_one(ic, oc, ao_tiles)

            def outproj_strip(ic, ao_tiles):
                for sub in range(ICH // P):
                    outproj(ic * (ICH // P) + sub, ao_tiles)

            if streaming:
                nstrips = N // ICH if MAX_STRIPS is None else MAX_STRIPS
                pending = None               # (ic, ao_tiles) awaiting outproj

                def alloc_q():
                    return [qpool.tile([P, QCH], BF16, tag=f"qs{mc}",
                                       name=f"qs{mc}")
                            for mc in range(PAIRS)]

                # prologue: project strip 0
                proj_v_strip(0, strip=strip0_v, vset=0)
                q_cur = alloc_q()
                proj_qk_strip(xqT, wq, 0, lambda mc: q_cur[mc],
                              strip=strip0_q)
                proj_qk_strip(xkT, wk, 0,
                              lambda mc: kTs[0][mc][:, 0:ICH],
                              strip=strip0_k)
                # steady state: attention for strip ic interleaved with the
                # projections of the next strip, so the Act engine (exp, the
                # per-strip bottleneck) never drains
                total = REPEAT * nstrips
                for it in range(total):
                    ic = it % nstrips
                    cset = (it // nstrips) % 2
                    nic = (it + 1) % nstrips if it + 1 < total else None
                    nset = ((it + 1) // nstrips) % 2
                    ao_tiles = [aopool.tile([P, QCH], BF16,
                                            tag=f"aos{pp}", name=f"aos{pp}")
                                for pp in range(PAIRS)]
                    q_next = None
                    # prefetch next strip's x loads at the top of the
                    # iteration so the DMAs run during the attention pairs
                    # instead of just-in-time before the projections
                    if nic is not None:
                        ns_v = load_strip(xvT, nic, "pf_v", split=2)
                        ns_q = load_strip(xqT, nic, "pf_q", split=2)
                        ns_k = load_strip(xkT, nic, "pf_k", split=2)
                        q_next = alloc_q()
                    # filler chains: out-projection of the previous strip and
                    # projections of the next strip, round-robin interleaved
                    # between attention score/AV blocks so the PE always has
                    # ready work while the Act engine paces the exp stream
                    phase_lists = []
                    if pending is not None:
                        pic, paos = pending
                        phase_lists.append([
                            (lambda s=sub, o=oc: outproj_one(
                                pic * (ICH // P) + s, o, paos))
                            for sub in range(ICH // P)
                            for oc in range(HID // 512)])
                    if nic is not None:
                        qn = q_next
                        phase_lists.append([
                            (lambda s=sub: proj_v_sub(nic, s, ns_v, nset))
                            for sub in range(ICH // P)])
                        phase_lists.append([
                            (lambda m=mc: proj_qk_mc(
                                wq, nic, m, lambda mm: qn[mm], ns_q))
                            for mc in range(PAIRS)])
                        phase_lists.append([
                            (lambda m=mc: proj_qk_mc(
                                wk, nic, m,
                                lambda mm: kTs[nset][mm][
                                    :, nic * ICH:(nic + 1) * ICH], ns_k))
                            for mc in range(PAIRS)])
                    chains = [t for tup in zip_longest(*phase_lists)
                              for t in tup if t is not None]
                    nblk = 4 * len([kc for kc in range(NKC)
                                    if plan[(ic, kc)][0] != "skip"])
                    state = [0, 0]  # blocks seen, chains emitted

                    def stepper():
                        state[0] += 1
                        tgt = len(chains) * state[0] // max(nblk, 1)
                        while state[1] < tgt:
                            chains[state[1]]()
                            state[1] += 1

                    for pp in range(PAIRS):
                        attn_block(pp, ic, q_cur[pp], ao_tiles[pp], cset,
                                   stepper=stepper)
                    while state[1] < len(chains):
                        chains[state[1]]()
                        state[1] += 1
                    pending = (ic, ao_tiles)
                    q_cur = q_next
                outproj_strip(*pending)
            else:
                for _rep in range(REPEAT):
                    for ic in range(N // ICH):
                        pre = _rep == 0 and ic == 0
                        proj_v_strip(ic, strip=strip0_v if pre else None)
                        proj_qk_strip(
                            xqT, wq, ic,
                            lambda mc: qT[mc][:, ic * ICH:(ic + 1) * ICH],
                            strip=strip0_q if pre else None)
                        proj_qk_strip(
                            xkT, wk, ic,
                            lambda mc: kTs[0][mc][:, ic * ICH:(ic + 1) * ICH],
                            strip=strip0_k if pre else None)
                    for qc in range(NQC):
                        ao_tiles = [aopool.tile([P, QCH], BF16,
                                                tag=f"aos{pp}", name=f"aos{pp}")
                                    for pp in range(PAIRS)]
                        for pp in range(PAIRS):
                            attn_block(pp, qc,
                                       qT[pp][:, qc * QCH:(qc + 1) * QCH],
                                       ao_tiles[pp])
                        outproj_strip(qc, ao_tiles)

    nc.compile()
    return nc, masks_np


def _get_nc(is_causal, start_pos):
    key = (bool(is_causal), int(start_pos), REPEAT, MAX_STRIPS, ABLATE)
    if key not in _NC_CACHE:
        _NC_CACHE[key] = _build_nc(bool(is_causal), int(start_pos))
    return _NC_CACHE[key]


# ---------------------------------------------------------------- entry
def kernel(x_q, x_k, x_v, W_q, W_k, W_v, W_out, padding_mask, is_causal,
           start_pos):
    x_q = np.asarray(x_q, dtype=np.float32)
    x_k = np.asarray(x_k, dtype=np.float32)
    x_v = np.asarray(x_v, dtype=np.float32)
    W_q = np.asarray(W_q, dtype=np.float32)
    W_k = np.asarray(W_k, dtype=np.float32)
    W_v = np.asarray(W_v, dtype=np.float32)
    W_out = np.asarray(W_out, dtype=np.float32)
    padding_mask = np.asarray(padding_mask).astype(bool)
    is_causal = int(np.asarray(is_causal))
    start_pos = int(np.asarray(start_pos))

    nc, masks = _get_nc(is_causal, start_pos)

    cos2, sin2 = _rope_tables()

    in_maps = []
    for c in range(NCORES):
        bi, hg = divmod(c, GROUPS)
        hs = hg * HD
        kpad = np.ascontiguousarray(
            padding_mask[bi].astype(np.float32).reshape(NKC, P).T
        ).astype(BF16NP)
        in_maps.append({
            "xqT": np.ascontiguousarray(x_q[bi].T).astype(BF16NP),
            "xkT": np.ascontiguousarray(x_k[bi].T).astype(BF16NP),
            "xvT": np.ascontiguousarray(x_v[bi].T).astype(BF16NP),
            "wqT": np.ascontiguousarray(W_q[hs:hs + HD].T).astype(BF16NP),
            "wkT": np.ascontiguousarray(W_k[hs:hs + HD].T).astype(BF16NP),
            "wvT": np.ascontiguousarray(W_v[hs:hs + HD].T).astype(BF16NP),
            "woT": np.ascontiguousarray(W_out[:, hs:hs + HD].T).astype(BF16NP),
            "cos": cos2.astype(BF16NP),
            "sin": sin2.astype(BF16NP),
            "masks": masks.astype(BF16NP),
            "kpad": kpad,
        })

    res = run_bass_kernel_spmd(nc, in_maps, list(range(NCORES)))
    out = np.empty((B, N, HID), dtype=np.float32)
    for bi in range(B):
        out[bi] = res.results[GROUPS * bi]["y"]
        for g in range(1, GROUPS):
            out[bi] += res.results[GROUPS * bi + g]["y"]
    return out



# revision 27
# speedup vs baseline: 1.4439x; 1.3647x over previous
"""Multi-head attention (RoPE, causal) Trainium2 Bass kernel.

Sharding (8 cores): data-parallel over batch (4) x tensor-parallel over
heads (16 -> 2 groups of 8).  Core c handles batch c//2 and head group
c%2.  Attention is fully head-local; the out-projection partial sums of
the two head groups of each batch are added on the host.

Per-core device kernel (sizes hardcoded for b=4, n=2048, hidden=1024,
h=16, d=64).  All matmuls in bf16 with fp32 PSUM accumulation (an fp8
DoubleRow variant was 2x faster on the PE but exceeded the 2e-2 error
budget; any single fp8 stage already costs >=4e-2 max-rel error):
  - QKV projections as bf16 matmuls; x and W arrive host-pre-transposed
    (c-major) and pre-cast, so no on-device transposes.
  - RoPE in head-transposed layout: u = q*sinP on DVE (sinP pair-swapped
    on the host so the +-1 pair-permutation matmul PM @ u on the PE
    directly yields rot(q)*sin), then q*cos + that on DVE; all DVE
    operands are bf16 SBUF so the 2x mode applies.
  - Scores computed transposed, s_T[k, q], two heads of a pair row-tiled
    onto the PE at tile_position (0,0)/(64,0), into one [128, 2, 512]
    PSUM tile so a single Exp activation (scale=1/8 folded in) covers
    both heads.  Diagonal blocks compute only the valid q column range
    [vs:512); the <=128-wide partial triangle is zeroed by a bf16 mask
    multiply on the otherwise-idle GpSimd engine.
  - AV: v augmented with 64 pad-ones columns (M=128) so PSUM rows
    64:128 hold the softmax denominator already broadcast across 64
    partitions; normalization is reciprocal+mul on DVE.
  - Software-pipelined strips: the projections of strip ic+1 are
    interleaved between the four attention pairs of strip ic so the Act
    engine (exp) never drains; the out-projection is deferred one strip
    so DVE normalization never stalls the PE.
"""

from itertools import zip_longest

import numpy as np
import ml_dtypes

import concourse.bass as bass
import concourse.mybir as mybir
from concourse import bacc
from concourse import hw_specs as _hw_specs
from concourse.tile import TileContext
from concourse.bass_utils import run_bass_kernel_spmd

# Calibrate the Tile scheduler's cost model to rates measured on this HW
# (microbenchmarks, steady-state chains): bf16 matmuls stream ~2 columns
# per cycle (F=512 chain ~150 ns vs the 213 ns 1-col/cycle default), the
# Act exp and DVE ops run ~0.7-0.8x the default cost, and GpSimd 2-input
# ops are ~2x SLOWER than the default.  A closer model gives the static
# per-engine schedule fewer runtime stalls.  Must run before the first
# compile in the process: the Rust cost model snapshots these class attrs
# into a process-global OnceCell on first use.
_hw_specs.TRN2Spec.PE_CYCLE = 1e9 / 3.2e9
_hw_specs.TRN2Spec.PE_CYCLE_PSTATE_MID = 1e9 / 1.6e9
_hw_specs.TRN2Spec.CYCLE_T = {
    **_hw_specs.TRN2Spec.CYCLE_T,
    mybir.EngineType.DVE: 0.70,
    mybir.EngineType.Activation: 0.90,
    mybir.EngineType.Pool: 1.67,
}

# ---------------------------------------------------------------- constants
B, N, HID = 4, 2048, 1024
H = 16
D = HID // H                     # 64
NCORES = 8
GROUPS = NCORES // B             # 2 head groups
HPG = H // GROUPS                # 8 heads per core
HD = HPG * D                     # 512 local head dims
PAIRS = HPG // 2                 # 4 head pairs per core
ROPE_THETA = 10000.0
SCALE = 0.125                    # 1/sqrt(d)
WSCALE = 1.0

P = 128
CC = HID // P                    # 8 contraction chunks for projections
ICH = 512                        # projection i-chunk (moving free dim)
QCH = 512                        # attention q-chunk
KCH = 128                        # attention k-chunk
NQC = N // QCH                   # 4
NKC = N // KCH                   # 16

F32 = mybir.dt.float32
BF16 = mybir.dt.bfloat16
FP8 = mybir.dt.float8e4
BF16NP = ml_dtypes.bfloat16
FP8NP = ml_dtypes.float8_e4m3
DR = mybir.MatmulPerfMode.DoubleRow

_NC_CACHE = {}
MAX_STRIPS = None
REPEAT = 1
ABLATE = frozenset()     # timing ablations: {"attn","proj","outproj","xdma"}


# ---------------------------------------------------------------- host prep
def _allow_matrix(is_causal, start_pos):
    i = np.arange(N)[:, None]    # query index
    j = np.arange(N)[None, :]    # key index
    if is_causal:
        return (j < start_pos) | ((i >= start_pos) & (i >= j))
    return np.ones((N, N), dtype=bool)


def _block_plan(is_causal, start_pos):
    """Classify each (qc, kc) score block.

    plan[(qc, kc)] is (kind, var, vs, ms, me):
      kind: 'skip' | 'full' | 'partial'
      var:  mask variant index (partial only)
      vs:   first valid q column in the 512-wide chunk (block computed
            over [vs:512) only)
      [ms, me): q column range where the partial triangle needs masking
    """
    allow = _allow_matrix(is_causal, start_pos)
    plan = {}
    variants = []
    vkeys = {}
    for qc in range(NQC):
        for kc in range(NKC):
            blk = allow[qc * QCH:(qc + 1) * QCH, kc * KCH:(kc + 1) * KCH]
            if not blk.any():
                plan[(qc, kc)] = ("skip", None, 0, 0, 0)
                continue
            if blk.all():
                plan[(qc, kc)] = ("full", None, 0, 0, 0)
                continue
            bT = blk.T               # [128 k, 512 q]
            start = np.argmax(bT, axis=1)
            for r in range(KCH):
                if not bT[r].any():
                    raise NotImplementedError("empty k-row in partial block")
                s = start[r]
                if not bT[r, s:].all() or bT[r, :s].any():
                    raise NotImplementedError("non-suffix mask row")
            key = start.tobytes()
            if key not in vkeys:
                vkeys[key] = len(variants)
                variants.append(start.astype(np.float32))
            vs = int(start.min())
            me = int(start.max())
            plan[(qc, kc)] = ("partial", vkeys[key], vs, vs, me)
    if not variants:
        variants.append(np.zeros(KCH, dtype=np.float32))
    # sanity: first non-skip block per qc row must cover the full q range
    for qc in range(NQC):
        for kc in range(NKC):
            kind, _, vs, _, _ = plan[(qc, kc)]
            if kind != "skip":
                assert vs == 0, f"first block of row {qc} has vs={vs}"
                break
    # If every variant is a slope-1 diagonal start[k] = k + o (the causal
    # case), all masks are column slices of ONE [128, QCH + o_max] ramp
    # R[k, t] = (t >= k + o_max): variant o lives at column offset
    # o_max - o.  That shrinks the SBUF table ~2.3x vs concatenation.
    offs = []
    for v in variants:
        o = int(v[0])
        if np.array_equal(v, np.arange(KCH) + o):
            offs.append(o)
        else:
            offs.append(None)
    if all(o is not None for o in offs) and variants:
        o_max = max(offs)
        k = np.arange(KCH)[:, None]
        t = np.arange(QCH + o_max)[None, :]
        masks = (t >= k + o_max).astype(np.float32)   # [128, QCH + o_max]
        for key in plan:
            kind, var, vs, ms, me = plan[key]
            if kind == "partial":
                plan[key] = (kind, o_max - offs[var], vs, ms, me)
    else:
        # fallback: concatenated per-variant blocks at offsets var*QCH
        q = np.arange(QCH)[None, :]
        blocks = [(q >= v[:, None]).astype(np.float32) for v in variants]
        masks = np.concatenate(blocks, axis=1)        # [128, V*QCH]
        for key in plan:
            kind, var, vs, ms, me = plan[key]
            if kind == "partial":
                plan[key] = (kind, var * QCH, vs, ms, me)
    return plan, masks


def _rope_tables():
    inv_freq = 1.0 / (ROPE_THETA ** (np.arange(0, D, 2, dtype=np.float64) / D))
    t = np.arange(N, dtype=np.float64)
    freqs = t[:, None] * inv_freq[None, :]        # [N, 32]
    freqs = np.repeat(freqs, 2, axis=1)           # [N, 64]
    cos = np.cos(freqs).T.astype(np.float32)      # [64, N]
    sin = np.sin(freqs).T.astype(np.float32)
    # sinN folds the rotate-half signs: dst = raw*cos + pairswap(raw)*sinN
    # with sinN[2r] = -sin[2r], sinN[2r+1] = +sin[2r+1]; the pair swap is a
    # partition-strided SBUF->SBUF DMA so no PE/PSUM is involved.
    sinN = sin.copy()
    sinN[0::2] = -sin[0::2]
    # duplicate rows so both heads of a pair (partitions 0:64 / 64:128)
    # see the same table at matching partition base
    cos2 = np.concatenate([cos, cos], axis=0)     # [128, N]
    sin2 = np.concatenate([sinN, sinN], axis=0)
    return np.ascontiguousarray(cos2), np.ascontiguousarray(sin2)


# ---------------------------------------------------------------- device IR
def _build_nc(is_causal, start_pos):
    plan, masks_np = _block_plan(is_causal, start_pos)
    mwid = masks_np.shape[1]
    streaming = bool(is_causal)
    exp_scale = SCALE

    nc = bacc.Bacc("TRN2", target_bir_lowering=False, debug=False)

    xqT = nc.declare_dram_parameter("xqT", [HID, N], BF16, isOutput=False).ap()
    xkT = nc.declare_dram_parameter("xkT", [HID, N], BF16, isOutput=False).ap()
    xvT = nc.declare_dram_parameter("xvT", [HID, N], BF16, isOutput=False).ap()
    wqT = nc.declare_dram_parameter("wqT", [HID, HD], BF16, isOutput=False).ap()
    wkT = nc.declare_dram_parameter("wkT", [HID, HD], BF16, isOutput=False).ap()
    wvT = nc.declare_dram_parameter("wvT", [HID, HD], BF16, isOutput=False).ap()
    woT = nc.declare_dram_parameter("woT", [HD, HID], BF16, isOutput=False).ap()
    cos_d = nc.declare_dram_parameter("cos", [P, N], BF16, isOutput=False).ap()
    sin_d = nc.declare_dram_parameter("sin", [P, N], BF16, isOutput=False).ap()
    msk_d = nc.declare_dram_parameter("masks", [P, mwid], BF16,
                                      isOutput=False).ap()
    kpad_d = nc.declare_dram_parameter("kpad", [P, NKC], BF16, isOutput=False).ap()
    y = nc.declare_dram_parameter("y", [N, HID], F32, isOutput=True).ap()

    with TileContext(nc) as tc:
        with (
            tc.tile_pool(name="const", bufs=1) as const,
            tc.tile_pool(name="persist", bufs=1) as persist,
            tc.tile_pool(name="xstrip", bufs=3) as xpool,
            tc.tile_pool(name="qpool", bufs=2) as qpool,
            tc.tile_pool(name="aopool", bufs=2) as aopool,
            tc.tile_pool(name="work", bufs=2) as work,
            tc.tile_pool(name="ppool", bufs=6) as ppool,
            tc.tile_pool(name="psmm", bufs=2, space="PSUM") as psmm,
            tc.tile_pool(name="pssc", bufs=2, space="PSUM") as pssc,
            tc.tile_pool(name="psav", bufs=2, space="PSUM") as psav,
        ):
            # ---------------- weights first: the first projections need them
            wv = const.tile([P, CC, HD], BF16, tag="wv", name="wv")
            nc.sync.dma_start(out=wv, in_=wvT.rearrange("(cc p) m -> p cc m", p=P))
            wq = const.tile([P, CC, HD], BF16, tag="wq", name="wq")
            nc.sync.dma_start(out=wq, in_=wqT.rearrange("(cc p) m -> p cc m", p=P))

            # strip-0 x loads, issued before the rest of the tables
            def load_strip(x_dram, ic, nm, split=1):
                strip = xpool.tile([P, CC, ICH], BF16, tag="xstrip", name=nm)
                src = x_dram.rearrange("(cc p) n -> p cc n", p=P)[
                    :, :, ic * ICH:(ic + 1) * ICH]
                step = CC // split
                if "xdma" not in ABLATE:
                    for s in range(split):
                        nc.sync.dma_start(
                            out=strip[:, s * step:(s + 1) * step],
                            in_=src[:, s * step:(s + 1) * step])
                return strip

            strip0_v = load_strip(xvT, 0, "strip0v")
            strip0_q = load_strip(xqT, 0, "strip0q")
            wk = const.tile([P, CC, HD], BF16, tag="wk", name="wk")
            nc.sync.dma_start(out=wk, in_=wkT.rearrange("(cc p) m -> p cc m", p=P))
            strip0_k = load_strip(xkT, 0, "strip0k")

            # ---------------- remaining constants / tables
            cost = const.tile([P, N], BF16, tag="cos", name="cos")
            sint = const.tile([P, N], BF16, tag="sin", name="sin")
            nc.sync.dma_start(out=cost, in_=cos_d)
            nc.sync.dma_start(out=sint, in_=sin_d)
            mskt = const.tile([P, mwid], BF16, tag="masks", name="mskt")
            nc.sync.dma_start(out=mskt, in_=msk_d)
            kpad = const.tile([P, NKC], BF16, tag="kpad", name="kpad")
            nc.sync.dma_start(out=kpad, in_=kpad_d)
            wo = const.tile([P, PAIRS, HID], BF16, tag="wo", name="wo")
            nc.sync.dma_start(out=wo, in_=woT.rearrange("(jc p) o -> p jc o", p=P))

            # ---------------- persistent activations
            # roped k per pair, double-buffered across repeats so the
            # next repeat's projections never wait on this one's attention
            kTs = [[persist.tile([P, N], BF16, tag=f"kT{s}{p}",
                                 name=f"kT{s}{p}")
                    for p in range(PAIRS)] for s in range(2)]
            kT = kTs[0]
            # v: per (head, k-chunk) blocks of [v(64) | pad-ones(64)]
            vtalls = [persist.tile([P, HPG, NKC, P], BF16, tag=f"vt{s}",
                                   name=f"vt{s}") for s in range(2)]
            vts = [[va[:, h] for h in range(HPG)] for va in vtalls]
            vtall, vt = vtalls[0], vts[0]
            if streaming:
                qT = None
            else:
                qT = [persist.tile([P, N], BF16, tag=f"qT{p}", name=f"qT{p}")
                      for p in range(PAIRS)]

            # pad-ones halves of vt, written once on the (idle) gpsimd engine
            for va in vtalls:
                for kc in range(NKC):
                    nc.gpsimd.tensor_copy(
                        out=va[:, :, kc, D:P],
                        in_=kpad[:, kc:kc + 1].to_broadcast([P, HPG, D]),
                    )

            def proj_qk_mc(w_sb, ic, mc, dst_of_mc, strip):
                """One q/k projection chain (pair mc of a 512-wide strip)."""
                if "proj" in ABLATE:
                    nc.vector.tensor_copy(out=dst_of_mc(mc),
                                          in_=cost[:, 0:ICH])
                    return
                isl = slice(ic * ICH, (ic + 1) * ICH)
                ps = psmm.tile([P, ICH], F32, tag="mm", name="pjmm")
                for cc in range(CC):
                    nc.tensor.matmul(
                        ps,
                        lhsT=w_sb[:, cc, mc * P:(mc + 1) * P],
                        rhs=strip[:, cc, :],
                        start=(cc == 0),
                        stop=(cc == CC - 1),
                    )
                raw = work.tile([P, ICH], BF16, tag="raw", name="raw")
                nc.vector.tensor_copy(out=raw, in_=ps)
                # rotate-half as a partition pair swap on the (idle) DMA
                # engines; the signs live in the sinN table
                rsw = work.tile([P, ICH], BF16, tag="rsw", name="rsw")
                nc.sync.dma_start(out=rsw[0:P - 1:2], in_=raw[1:P:2])
                nc.sync.dma_start(out=rsw[1:P:2], in_=raw[0:P - 1:2])
                u = work.tile([P, ICH], BF16, tag="ropeu", name="u")
                nc.vector.tensor_mul(u, rsw, sint[:, isl])
                cw = work.tile([P, ICH], BF16, tag="ropecw", name="cw")
                nc.vector.tensor_mul(cw, raw, cost[:, isl])
                nc.vector.tensor_add(dst_of_mc(mc), cw, u)

            def proj_qk_strip(x_dram, w_sb, ic, dst_of_mc, strip=None):
                """Project one 512-wide strip of q or k (all pairs) + RoPE."""
                if strip is None:
                    strip = load_strip(x_dram, ic, "strip")
                for mc in range(PAIRS):
                    proj_qk_mc(w_sb, ic, mc, dst_of_mc, strip)

            def proj_v_sub(ic4, sub, strip, vset):
                """Project one 128-wide sub-chunk of a v strip into vt."""
                if "proj" in ABLATE:
                    return
                vta = vtalls[vset]
                ic = ic4 * (ICH // P) + sub
                ps = psmm.tile([P, HD], F32, tag="mm", name="pvmm")
                for cc in range(CC):
                    nc.tensor.matmul(
                        ps,
                        lhsT=strip[:, cc, sub * P:(sub + 1) * P],
                        rhs=wv[:, cc, :],
                        start=(cc == 0),
                        stop=(cc == CC - 1),
                    )
                padb = kpad[:, ic:ic + 1]
                nc.vector.tensor_mul(
                    vta[:, :, ic, 0:D],
                    ps.rearrange("p (h d) -> p h d", h=HPG),
                    padb.to_broadcast([P, HPG, D]),
                )

            def proj_v_strip(ic4, strip=None, vset=0):
                """Project one 512-wide strip of v into vt (pad-scaled)."""
                if strip is None:
                    strip = load_strip(xvT, ic4, "vstrip")
                for sub in range(ICH // P):
                    proj_v_sub(ic4, sub, strip, vset)

            def scores_block(pp, qc, kc, q_ap, pt_out, vs, kT):
                """Scores for both heads of pair pp on k chunk kc, exp'd
                into pt_out[:, :, vs:512) ([128, 2, 512] view)."""
                ksl = slice(kc * KCH, (kc + 1) * KCH)
                sp = pssc.tile([P, 2, QCH], F32, tag="sc", name="smm")
                # high priority: the exp->scores round trip paces the whole
                # attention stream (the score-PSUM slot frees when the exp
                # two blocks back completes).  Without the boost the PE
                # stream runs the same-sem-gated AV matmuls first and the
                # Act engine starves for ~600 ns every block.
                with tc.high_priority(offset=64):
                    for hh in range(2):
                        base = hh * D
                        nc.tensor.matmul(
                            sp[:, hh, vs:],
                            lhsT=kT[pp][base:base + D, ksl],
                            rhs=q_ap[base:base + D, vs:],
                            start=True, stop=True,
                            tile_position=(base, 0),
                        )
                    nc.scalar.activation(
                        pt_out[:, :, vs:], sp[:, :, vs:],
                        mybir.ActivationFunctionType.Exp,
                        scale=exp_scale)

            def attn_block(pp, qc, q_ap, ao_tile, aset=0, stepper=None):
                kT, vt = kTs[aset], vts[aset]
                """Attention for head pair pp over q chunk qc.

                q_ap: [128, 2, 512] fp8 (chunk 1 zeroed)
                ao_tile: [128, 512] bf16 output (normalized attn @ v)
                """
                kcs = [kc for kc in range(NKC) if plan[(qc, kc)][0] != "skip"]
                if not kcs or "attn" in ABLATE:
                    if "attn" in ABLATE:
                        nc.vector.tensor_copy(out=ao_tile, in_=cost[:, 0:QCH])
                    if stepper is not None:
                        for _ in kcs:
                            stepper()
                    return
                avt = psav.tile([P, QCH], F32, tag="av", name="avA")
                avt2 = psav.tile([P, QCH], F32, tag="av", name="avB")
                avs = (avt, avt2)
                for avi, kc in enumerate(kcs):
                    kind, var, vs, ms, me = plan[(qc, kc)]
                    pt = ppool.tile([P, 2, QCH], BF16, tag="p", name="p")
                    scores_block(pp, qc, kc, q_ap, pt, vs, kT)
                    if kind == "partial" and me > ms and "mask" not in ABLATE:
                        # DVE, not GpSimd: measured Pool tensor_mul is ~2x
                        # slower than DVE and this hop sits between exp and
                        # the AV matmul on the critical path
                        with tc.high_priority(offset=32):
                            for hh in range(2):
                                nc.vector.tensor_mul(
                                    pt[:, hh, ms:me], pt[:, hh, ms:me],
                                    mskt[:, var + ms:var + me])
                    for hh in range(2):
                        nc.tensor.matmul(
                            avs[hh][:, vs:],
                            lhsT=vt[2 * pp + hh][:, kc, :],
                            rhs=pt[:, hh, vs:],
                            start=(avi == 0), stop=(avi == len(kcs) - 1),
                            skip_group_check=True,
                        )
                    if stepper is not None:
                        stepper()
                # rows 64:128 of avs hold the denominator broadcast across
                # 64 partitions (from the pad-ones columns of vt)
                if "norm" in ABLATE:
                    nc.vector.tensor_copy(out=ao_tile, in_=cost[:, 0:QCH])
                    return
                # reciprocal_approx_fast (~18 bits, ~5x faster than the
                # iterative reciprocal): the denominators are sums of exps in
                # [e^-5, 3e5], far from the undefined edge cases, and the
                # result feeds a bf16 multiply anyway.  High priority: the
                # next pair's AV accumulation waits on this pair's PSUM
                # banks, so the norm tail must not sit behind other DVE work.
                with tc.high_priority(offset=32):
                    for hh in range(2):
                        rec = work.tile([D, QCH], F32, tag="rec", name="rec")
                        # stage the denominators through SBUF: the custom
                        # recip uop mis-handles a direct PSUM read (observed
                        # a NaN), and the copy is cheap
                        nc.vector.tensor_copy(out=rec, in_=avs[hh][D:2 * D, :])
                        nc.vector.reciprocal_approx_fast(rec, rec)
                        nc.vector.tensor_mul(ao_tile[hh * D:(hh + 1) * D, :],
                                             avs[hh][0:D, :], rec)

            def outproj_one(ic, oc, ao_tiles):
                if "outproj" in ABLATE:
                    return
                isl = slice((ic % (ICH // P)) * P, (ic % (ICH // P)) * P + P)
                osl = slice(oc * 512, (oc + 1) * 512)
                ps = psmm.tile([P, 512], F32, tag="mm", name="yps")
                for pp in range(PAIRS):
                    nc.tensor.matmul(
                        ps,
                        lhsT=ao_tiles[pp][:, isl],
                        rhs=wo[:, pp, osl],
                        start=(pp == 0), stop=(pp == PAIRS - 1),
                    )
                yt = work.tile([P, 512], F32, tag="yout", name="yt")
                # drain on the Act engine (Copy shares the exp table set) so
                # the outproj chain doesn't queue behind DVE rope/norm work
                nc.scalar.activation(yt, ps, mybir.ActivationFunctionType.Copy)
                nc.sync.dma_start(
                    out=y[ic * P:(ic + 1) * P, osl], in_=yt)

            def outproj(ic, ao_tiles):
                for oc in range(HID // 512):
                    outproj_one(ic, oc, ao_tiles)

            def outproj_strip(ic, ao_tiles):
                for sub in range(ICH // P):
                    outproj(ic * (ICH // P) + sub, ao_tiles)

            if streaming:
                nstrips = N // ICH if MAX_STRIPS is None else MAX_STRIPS
                pending = None               # (ic, ao_tiles) awaiting outproj

                def alloc_q():
                    return [qpool.tile([P, QCH], BF16, tag=f"qs{mc}",
                                       name=f"qs{mc}")
                            for mc in range(PAIRS)]

                # prologue: project strip 0
                proj_v_strip(0, strip=strip0_v, vset=0)
                q_cur = alloc_q()
                proj_qk_strip(xqT, wq, 0, lambda mc: q_cur[mc],
                              strip=strip0_q)
                proj_qk_strip(xkT, wk, 0,
                              lambda mc: kTs[0][mc][:, 0:ICH],
                              strip=strip0_k)
                # steady state: attention for strip ic interleaved with the
                # projections of the next strip, so the Act engine (exp, the
                # per-strip bottleneck) never drains
                total = REPEAT * nstrips
                for it in range(total):
                    ic = it % nstrips
                    cset = (it // nstrips) % 2
                    nic = (it + 1) % nstrips if it + 1 < total else None
                    nset = ((it + 1) // nstrips) % 2
                    ao_tiles = [aopool.tile([P, QCH], BF16,
                                            tag=f"aos{pp}", name=f"aos{pp}")
                                for pp in range(PAIRS)]
                    q_next = None
                    # prefetch next strip's x loads at the top of the
                    # iteration so the DMAs run during the attention pairs
                    # instead of just-in-time before the projections
                    if nic is not None:
                        ns_v = load_strip(xvT, nic, "pf_v", split=2)
                        ns_q = load_strip(xqT, nic, "pf_q", split=2)
                        ns_k = load_strip(xkT, nic, "pf_k", split=2)
                        q_next = alloc_q()
                    # filler chains: out-projection of the previous strip and
                    # projections of the next strip, round-robin interleaved
                    # between attention score/AV blocks so the PE always has
                    # ready work while the Act engine paces the exp stream
                    phase_lists = []
                    if pending is not None:
                        pic, paos = pending
                        phase_lists.append([
                            (lambda s=sub, o=oc: outproj_one(
                                pic * (ICH // P) + s, o, paos))
                            for sub in range(ICH // P)
                            for oc in range(HID // 512)])
                    if nic is not None:
                        qn = q_next
                        phase_lists.append([
                            (lambda s=sub: proj_v_sub(nic, s, ns_v, nset))
                            for sub in range(ICH // P)])
                        phase_lists.append([
                            (lambda m=mc: proj_qk_mc(
                                wq, nic, m, lambda mm: qn[mm], ns_q))
                            for mc in range(PAIRS)])
                        phase_lists.append([
                            (lambda m=mc: proj_qk_mc(
                                wk, nic, m,
                                lambda mm: kTs[nset][mm][
                                    :, nic * ICH:(nic + 1) * ICH], ns_k))
                            for mc in range(PAIRS)])
                    chains = [t for tup in zip_longest(*phase_lists)
                              for t in tup if t is not None]
                    nblk = 4 * len([kc for kc in range(NKC)
                                    if plan[(ic, kc)][0] != "skip"])
                    state = [0, 0]  # blocks seen, chains emitted

                    def stepper():
                        state[0] += 1
                        tgt = len(chains) * state[0] // max(nblk, 1)
                        while state[1] < tgt:
                            chains[state[1]]()
                            state[1] += 1

                    for pp in range(PAIRS):
                        attn_block(pp, ic, q_cur[pp], ao_tiles[pp], cset,
                                   stepper=stepper)
                    while state[1] < len(chains):
                        chains[state[1]]()
                        state[1] += 1
                    pending = (ic, ao_tiles)
                    q_cur = q_next
                outproj_strip(*pending)
            else:
                for _rep in range(REPEAT):
                    for ic in range(N // ICH):
                        pre = _rep == 0 and ic == 0
                        proj_v_strip(ic, strip=strip0_v if pre else None)
                        proj_qk_strip(
                            xqT, wq, ic,
                            lambda mc: qT[mc][:, ic * ICH:(ic + 1) * ICH],
                            strip=strip0_q if pre else None)
                        proj_qk_strip(
                            xkT, wk, ic,
                            lambda mc: kTs[0][mc][:, ic * ICH:(ic + 1) * ICH],
                            strip=strip0_k if pre else None)
                    for qc in range(NQC):
                        ao_tiles = [aopool.tile([P, QCH], BF16,
                                                tag=f"aos{pp}", name=f"aos{pp}")
                                    for pp in range(PAIRS)]
                        for pp in range(PAIRS):
                            attn_block(pp, qc,
                                       qT[pp][:, qc * QCH:(qc + 1) * QCH],
                                       ao_tiles[pp])
                        outproj_strip(qc, ao_tiles)

    nc.compile()
    return nc, masks_np


def _get_nc(is_causal, start_pos):
    key = (bool(is_causal), int(start_pos), REPEAT, MAX_STRIPS, ABLATE)
    if key not in _NC_CACHE:
        _NC_CACHE[key] = _build_nc(bool(is_causal), int(start_pos))
    return _NC_CACHE[key]


# ---------------------------------------------------------------- entry
def kernel(x_q, x_k, x_v, W_q, W_k, W_v, W_out, padding_mask, is_causal,
           start_pos):
    x_q = np.asarray(x_q, dtype=np.float32)
    x_k = np.asarray(x_k, dtype=np.float32)
    x_v = np.asarray(x_v, dtype=np.float32)
    W_q = np.asarray(W_q, dtype=np.float32)
    W_k = np.asarray(W_k, dtype=np.float32)
    W_v = np.asarray(W_v, dtype=np.float32)
    W_out = np.asarray(W_out, dtype=np.float32)
    padding_mask = np.asarray(padding_mask).astype(bool)
    is_causal = int(np.asarray(is_causal))
    start_pos = int(np.asarray(start_pos))

    nc, masks = _get_nc(is_causal, start_pos)

    cos2, sin2 = _rope_tables()

    in_maps = []
    for c in range(NCORES):
        bi, hg = divmod(c, GROUPS)
        hs = hg * HD
        kpad = np.ascontiguousarray(
            padding_mask[bi].astype(np.float32).reshape(NKC, P).T
        ).astype(BF16NP)
        in_maps.append({
            "xqT": np.ascontiguousarray(x_q[bi].T).astype(BF16NP),
            "xkT": np.ascontiguousarray(x_k[bi].T).astype(BF16NP),
            "xvT": np.ascontiguousarray(x_v[bi].T).astype(BF16NP),
            "wqT": np.ascontiguousarray(W_q[hs:hs + HD].T).astype(BF16NP),
            "wkT": np.ascontiguousarray(W_k[hs:hs + HD].T).astype(BF16NP),
            "wvT": np.ascontiguousarray(W_v[hs:hs + HD].T).astype(BF16NP),
            "woT": np.ascontiguousarray(W_out[:, hs:hs + HD].T).astype(BF16NP),
            "cos": cos2.astype(BF16NP),
            "sin": sin2.astype(BF16NP),
            "masks": masks.astype(BF16NP),
            "kpad": kpad,
        })

    res = run_bass_kernel_spmd(nc, in_maps, list(range(NCORES)))
    out = np.empty((B, N, HID), dtype=np.float32)
    for bi in range(B):
        out[bi] = res.results[GROUPS * bi]["y"]
        for g in range(1, GROUPS):
            out[bi] += res.results[GROUPS * bi + g]["y"]
    return out

